# revision 17
# baseline (speedup 1.0000x reference)
"""Trainium2 Bass kernel for nn_DecompGen (conditional generator + rank-decomposed
outer-product head).

Sharding: pure data parallelism — batch B=256 is split 32-per-core across 8
NeuronCores; the small parameter set is replicated.  Training-mode BatchNorm
needs full-batch statistics, so the kernel does two tiny AllReduces (one for
the stage-A convs' stats, one for stage-B).  The first BN (on the input
linear) is handled by replicating that stage's trivially small matmul on
every core, which removes a third AllReduce.

Layout conventions (per core, channels on SBUF partitions):
  - conv activations are stored "t-major": tile column = t*32 + b
  - convT is computed as K shifted matmuls accumulating into one PSUM bank;
    each weight-tap k contributes to output window t in [k, k+L_in); windows
    are emitted as (covered + fresh) split matmuls so per-element PSUM
    has_written semantics stay uniform per instruction.
  - the rank-weighted outer-product head runs per-sample matmuls
    out[(ci,hi), wi] = sum_r (coef*c)[r,ci]*h[r,hi] . w[r,wi]
"""

import os
import threading

import numpy as np

import concourse.bacc as bacc
import concourse.mybir as mybir
import concourse.tile as tile
from concourse.bass_utils import run_bass_kernel_spmd

F32 = mybir.dt.float32
F32R = mybir.dt.float32r
BF16 = mybir.dt.bfloat16

N_CORES = 8
B, NOISE, NCLASS, RANK = 256, 100, 10, 512
R4, R2 = RANK // 4, RANK // 2  # 128, 256
BC = B // N_CORES  # 32 samples per core
EPS = 1e-5

# precision config: "f32" (exact), "f32r" (fp32 storage, fast PE mode on the
# big convs), "wbf16" (bf16 weights + f32r activations), "bf16" (bf16
# weights/activations on the conv + einsum path)
PREC = os.environ.get("KERNEL_PREC", "f32r")
NO_AR = os.environ.get("KERNEL_NO_AR", "0") == "1"
SIM_SAFE = os.environ.get("KERNEL_SIM_SAFE", "0") == "1"

_lock = threading.Lock()
_cache: dict = {}


def _cdt():
    """storage dtype of conv WEIGHTS"""
    if PREC in ("bf16", "wbf16"):
        return BF16
    if PREC == "f32r":
        return F32R
    return F32


def _adt():
    """storage dtype of conv ACTIVATIONS (matmul moving operands)"""
    if PREC == "bf16":
        return BF16
    if PREC in ("f32r", "wbf16"):
        return F32R
    return F32


def _mmbc(ap):
    return ap


# --------------------------------------------------------------------------
# device kernel
# --------------------------------------------------------------------------

def _emit(nc, tc, I, out_ap):
    ADT = _adt()          # conv activations (h1/h2, latent)
    WDT = _cdt()          # conv weights (host already cast)
    GDT = BF16   # einsum G / w3 path (always bf16: feeds fp32-accum matmuls)
    Alu = mybir.AluOpType
    Act = mybir.ActivationFunctionType

    sb = tc.alloc_tile_pool(name="sb", bufs=1)
    # all PSUM tiles are <= one bank; share 8 rotating bank slots
    psp = tc.alloc_tile_pool(name="ps", bufs=8, space="PSUM")
    dram = tc.alloc_tile_pool(name="dram", bufs=1, space="DRAM")

    def load(name, shape, dtype=F32, eng=None, chunks=1):
        t = sb.tile(list(shape), dtype, name=f"sb_{name}")
        eng = eng or nc.sync
        n = shape[1]
        step = (n + chunks - 1) // chunks
        for c0 in range(0, n, step):
            c1 = min(n, c0 + step)
            eng.dma_start(t[:, c0:c1], I[name][:, c0:c1])
        return t

    # ---- input loads (small/latency-critical first; big weights chunked so
    # the first matmuls of each conv can start before the tail arrives) ----
    onehot = load("onehot", (NCLASS, B))
    lin_wt = load("lin_wt", (NOISE, 128))
    emb_w = load("emb_w", (NCLASS, 128))
    noise_t = load("noise_t", (NOISE, B))
    bn0_g = load("bn0_g", (128, 1))
    bn0_b = load("bn0_b", (128, 1))
    gA = load("gA", (128, 6))
    beA = load("beA", (128, 6))
    coef_p = load("coef_p", (128, 4))
    gB = load("gB", (128, 4))
    beB = load("beB", (128, 4))
    b3h = load("b3h", (128, 4))
    b3w = load("b3w", (128, 4))
    wA_h = load("wA_h", (128, 2 * 16 * 128), WDT, chunks=4)
    wA_w = load("wA_w", (128, 2 * 16 * 128), WDT, eng=nc.scalar, chunks=4)
    wA_c = load("wA_c", (128, 2 * 3 * 4 * 128), WDT, eng=nc.gpsimd, chunks=2)
    wB_h = load("wB_h", (128, 16 * 2 * 128), WDT, chunks=4)
    wB_w = load("wB_w", (128, 16 * 2 * 128), WDT, eng=nc.scalar, chunks=4)
    wC_h = load("wC_h", (128, 2 * 2 * 4 * 128), WDT, eng=nc.gpsimd, chunks=2)
    wC_w = load("wC_w", (128, 2 * 2 * 4 * 128), WDT, eng=nc.scalar, chunks=2)

    eps_c = sb.tile([128, 1], F32, name="eps_c")
    nc.vector.memset(eps_c[:], EPS)
    invNB = sb.tile([128, 4], F32, name="invNB")
    nc.vector.memset(invNB[:], 1.0 / (B * 31))

    # ============== stage 0 + stage A (replicated, full batch) ============
    # Every core computes the full-batch pre-BN activations of the cheap
    # early stages, so all their BatchNorm statistics are local — no
    # AllReduce needed until stage B.  The host permutes the batch per core
    # (own 32 samples first), so "my shard" is always columns 0:32.
    ps_zf = psp.tile([128, B], F32, name="ps_zf", tag="ps")
    nc.tensor.matmul(ps_zf[:], lin_wt[:], noise_t[:], start=True, stop=True)
    ps_lab = psp.tile([128, B], F32, name="ps_lab", tag="ps")
    nc.tensor.matmul(ps_lab[:], emb_w[:], onehot[:], start=True, stop=True)

    st0 = sb.tile([128, 6], F32, name="st0")
    nc.vector.bn_stats(st0[:], ps_zf[:])
    mv0 = sb.tile([128, 2], F32, name="mv0")
    nc.vector.bn_aggr(mv0[:], st0[:])
    std0 = sb.tile([128, 1], F32, name="std0")
    nc.scalar.activation(std0[:], mv0[:, 1:2], Act.Sqrt, bias=eps_c[:])
    rstd0 = sb.tile([128, 1], F32, name="rstd0")
    nc.vector.reciprocal(rstd0[:], std0[:])
    A0 = sb.tile([128, 1], F32, name="A0")
    nc.vector.tensor_tensor(A0[:], rstd0[:], bn0_g[:], Alu.mult)
    t0_ = sb.tile([128, 1], F32, name="t0_")
    nc.vector.tensor_tensor(t0_[:], mv0[:, 0:1], A0[:], Alu.mult)
    B0 = sb.tile([128, 1], F32, name="B0")
    nc.vector.tensor_tensor(B0[:], bn0_b[:], t0_[:], Alu.subtract)

    # latent = [lrelu(bn0(z)) ; emb[label]]  (2 chunks of 128 ch, full batch)
    lat0 = sb.tile([128, B], ADT, name="lat0")
    nc.scalar.activation(lat0[:], ps_zf[:], Act.Identity, bias=B0[:], scale=A0[:])
    nc.vector.scalar_tensor_tensor(lat0[:], lat0[:], 0.01, lat0[:], Alu.mult, Alu.max)
    lat1 = sb.tile([128, B], ADT, name="lat1")
    nc.vector.tensor_copy(lat1[:], ps_lab[:])
    lat = [lat0, lat1]

    # ---- stage A convs (L_in=1): x[co, t, b] = sum_ci w[ci,co,t]*lat[ci,b]
    # full batch per k-block in PSUM; bn_stats per block (equal sizes) then
    # bn_aggr -> local full-batch (mean, var); own shard evacuated to SBUF.
    mvA = sb.tile([128, 12], F32, name="mvA")

    def stageA_conv(wtile, KA, G, kc, mvcol, name):
        outs = []
        for g in range(G):
            nblk = KA // kc
            stS = sb.tile([128, nblk * 6], F32, name=f"stA_{name}{g}")
            xdt = F32 if (ADT == BF16 or G > 1) else ADT
            x = sb.tile([128, KA * BC], xdt, name=f"xA_{name}{g}")
            for blk in range(nblk):
                ps = psp.tile([128, kc * B], F32, name=f"psA_{name}{g}_{blk}",
                              tag="ps")
                i = 0
                for kk in range(kc):
                    k = blk * kc + kk
                    for chunk in range(2):
                        idx = ((chunk * KA + k) * G + g) * 128
                        nc.tensor.matmul(ps[:, kk * B:(kk + 1) * B],
                                         wtile[:, idx:idx + 128], lat[chunk][:],
                                         start=(i == 0), stop=(i == 2 * kc - 1))
                        i += 1
                nc.vector.bn_stats(stS[:, blk * 6:(blk + 1) * 6], ps[:])
                src = ps[:].rearrange("p (kk b) -> p kk b", b=B)[:, :, 0:BC]
                dst = x[:].rearrange("p (t b) -> p t b", b=BC)[
                    :, blk * kc:(blk + 1) * kc, :]
                nc.scalar.copy(dst, src)
            nc.vector.bn_aggr(mvA[:, (mvcol + g) * 2:(mvcol + g) * 2 + 2], stS[:])
            outs.append(x)
        return outs

    xA_h = stageA_conv(wA_h, 16, 1, 2, 0, "h")[0]
    xA_w = stageA_conv(wA_w, 16, 1, 2, 1, "w")[0]
    xA_c = stageA_conv(wA_c, 3, 4, 1, 2, "c")

    # ---- per-channel BN coefficients: A = g/sqrt(var+eps), B = be - mean*A
    def bn_from_mv(mean_v, var_v, ncols, g_t, be_t, name):
        std = sb.tile([128, ncols], F32, name=f"std{name}")
        nc.scalar.activation(std[:], var_v, Act.Sqrt, bias=eps_c[:])
        rstd = sb.tile([128, ncols], F32, name=f"rstd{name}")
        nc.vector.reciprocal(rstd[:], std[:])
        Atl = sb.tile([128, ncols], F32, name=f"A{name}")
        nc.vector.tensor_tensor(Atl[:], rstd[:], g_t[:], Alu.mult)
        tmp = sb.tile([128, ncols], F32, name=f"tmp{name}")
        nc.vector.tensor_tensor(tmp[:], mean_v, Atl[:], Alu.mult)
        Btl = sb.tile([128, ncols], F32, name=f"B{name}")
        nc.vector.tensor_tensor(Btl[:], be_t[:], tmp[:], Alu.subtract)
        return Atl, Btl

    def bn_coeffs(stg, ncols, invN, g_t, be_t, name):
        sc = sb.tile([128, 2 * ncols], F32, name=f"sc{name}")
        nc.vector.tensor_tensor(sc[:], stg[:], invN[:], Alu.mult)
        mean = sc[:, 0:ncols]
        ex2 = sc[:, ncols:2 * ncols]
        msq = sb.tile([128, ncols], F32, name=f"msq{name}")
        nc.scalar.activation(msq[:], mean, Act.Square)
        var = sb.tile([128, ncols], F32, name=f"var{name}")
        nc.vector.tensor_tensor(var[:], ex2, msq[:], Alu.subtract)
        return bn_from_mv(mean, var[:], ncols, g_t, be_t, name)

    mvA_v = mvA[:].rearrange("p (c two) -> p c two", two=2)
    A_A, B_A = bn_from_mv(mvA_v[:, :, 0:1].squeeze(2), mvA_v[:, :, 1:2].squeeze(2),
                          6, gA, beA, "A")
    # fold the rank weights into the c-path coefficients (coef >= 0 commutes
    # with leaky-relu)
    nc.vector.tensor_tensor(A_A[:, 2:6], A_A[:, 2:6], coef_p[:], Alu.mult)
    nc.vector.tensor_tensor(B_A[:, 2:6], B_A[:, 2:6], coef_p[:], Alu.mult)

    def bn_apply(dst, src, Atl, Btl, col, slope):
        nc.scalar.activation(dst, src, Act.Identity,
                             bias=Btl[:, col:col + 1], scale=Atl[:, col:col + 1])
        nc.vector.scalar_tensor_tensor(dst, dst, slope, dst, Alu.mult, Alu.max)

    if ADT != BF16:
        h1, w1 = xA_h, xA_w
        bn_apply(h1[:], h1[:], A_A, B_A, 0, 0.2)
        bn_apply(w1[:], w1[:], A_A, B_A, 1, 0.2)
    else:
        h1 = sb.tile([128, 16 * BC], ADT, name="h1")
        bn_apply(h1[:], xA_h[:], A_A, B_A, 0, 0.2)
        w1 = sb.tile([128, 16 * BC], ADT, name="w1")
        bn_apply(w1[:], xA_w[:], A_A, B_A, 1, 0.2)
    # c~ = coef * lrelu(bn(c_pre)): written b-major (cols b*3+ci) for the head
    cT = []
    for g in range(4):
        ct = sb.tile([128, 3 * BC], F32, name=f"cT{g}")
        src = xA_c[g][:].rearrange("p (c b) -> p c b", b=BC)
        dstv = ct[:].rearrange("p (b c) -> p c b", c=3)
        bn_apply(dstv, src, A_A, B_A, 2 + g, 0.2)
        cT.append(ct)

    # ======================= stage B convs (16 -> 31) =====================
    # y[co, t, b] += sum_ci w2[ci, co, k] * h1[ci, t-k, b]

    def convT(wtile, src_chunks, LI, LO, KK, G, psname, n_bh=2, widx=None):
        """shifted-window convT: returns psum tiles [(g, bh) -> [128, LO*16]]

        Default emits one full-window matmul per (k, chunk) — on HW the
        per-element has_written bits make partially-overlapping windows
        accumulate correctly.  CoreSim asserts uniform pending state per
        matmul, so SIM_SAFE mode splits each k>=1 window into an
        all-covered part plus a single fresh output column.
        """
        pss = {}
        n_ck = len(src_chunks)
        for g in range(G):
            for bh in range(n_bh):
                ps = psp.tile([128, LO * 16], F32, name=f"ps{psname}_{g}_{bh}",
                              tag="ps")
                i = 0
                for k in range(KK):
                    for ck in range(n_ck):
                        lw = wtile[:, widx(ck, k, g):widx(ck, k, g) + 128]
                        src = src_chunks[ck]
                        last = (k == KK - 1 and ck == n_ck - 1)
                        if k == 0 or not SIM_SAFE:
                            rhs = src[:].rearrange("p (t b) -> p t b", b=BC)[
                                :, 0:LI, bh * 16:(bh + 1) * 16]
                            nc.tensor.matmul(ps[:, k * 16:(k + LI) * 16], lw, rhs,
                                             start=(i == 0), stop=last)
                            i += 1
                        else:
                            rhs = src[:].rearrange("p (t b) -> p t b", b=BC)[
                                :, 0:LI - 1, bh * 16:(bh + 1) * 16]
                            nc.tensor.matmul(
                                ps[:, k * 16:(k + LI - 1) * 16], lw, rhs,
                                start=False, stop=False)
                            i += 1
                            rhs2 = src[:, (LI - 1) * BC + bh * 16:
                                       (LI - 1) * BC + bh * 16 + 16]
                            nc.tensor.matmul(
                                ps[:, (k + LI - 1) * 16:(k + LI) * 16], lw, rhs2,
                                start=False, stop=last)
                            i += 1
                pss[(g, bh)] = ps
        return pss

    psB_h = convT(_W(wB_h), [_A(h1)], 16, 31, 16, 2, "Bh",
                  widx=lambda ck, k, g: (k * 2 + g) * 128)
    psB_w = convT(_W(wB_w), [_A(w1)], 16, 31, 16, 2, "Bw",
                  widx=lambda ck, k, g: (k * 2 + g) * 128)

    # evacuate + stats; xB tiles are [128, 31*BC] t-major over full b.
    # Each path (h, w) gets its own small AllReduce so the h-path collective
    # overlaps the w-path convs (and vice versa for stage C).
    def evacB(pss, name):
        stT = sb.tile([128, 8], F32, name=f"stB{name}")
        outs = []
        for g in range(2):
            xdt = F32 if ADT == BF16 else ADT
            x = sb.tile([128, 31 * BC], xdt, name=f"xB_{name}{g}")
            for bh in range(2):
                ps = pss[(g, bh)]
                col = g * 2 + bh
                dstv = x[:].rearrange("p (t b) -> p t b", b=BC)[:, :, bh * 16:(bh + 1) * 16]
                nc.vector.tensor_scalar(dstv, ps[:], 1.0, None, Alu.mult, Alu.add,
                                        accum_out=stT[:, col:col + 1])
                scr = sb.tile([128, 512], F32, name=f"scrB_{name}{g}{bh}",
                              tag="scr", bufs=2)
                nc.scalar.activation(scr[:, 0:31 * 16], ps[:], Act.Square,
                                     accum_out=stT[:, 4 + col:5 + col])
            outs.append(x)
        return outs, stT

    def allreduceB(stT, name):
        ar_in = dram.tile([128, 8], F32, name=f"arB{name}_in")
        ar_out = dram.tile([128, 8], F32, name=f"arB{name}_out")
        nc.sync.dma_start(ar_in[:], stT[:])
        if NO_AR:
            nc.gpsimd.dma_start(ar_out[:], ar_in[:])
        else:
            nc.gpsimd.collective_compute("AllReduce", Alu.add,
                                         replica_groups=[list(range(N_CORES))],
                                         ins=[ar_in.opt()], outs=[ar_out.opt()])
        stg = sb.tile([128, 8], F32, name=f"stBg{name}")
        nc.sync.dma_start(stg[:], ar_out[:])
        # combine the two batch-half partial sums: cols (s1 g0, s1 g1, s2 g0, s2 g1)
        sts = sb.tile([128, 4], F32, name=f"stBs{name}")
        v = stg[:].rearrange("p (c two) -> p c two", two=2)
        nc.vector.tensor_tensor(sts[:], v[:, :, 0:1].squeeze(2),
                                v[:, :, 1:2].squeeze(2), Alu.add)
        return sts

    def applyB(xs, A_t, B_t, name):
        outs = []
        for g in range(2):
            if ADT != BF16:
                t = xs[g]
                bn_apply(t[:], t[:], A_t, B_t, g, 0.2)
            else:
                t = sb.tile([128, 31 * BC], ADT, name=f"{name}2_{g}")
                bn_apply(t[:], xs[g][:], A_t, B_t, g, 0.2)
            outs.append(t)
        return outs

    xB_h, stBh = evacB(psB_h, "h")
    stsH = allreduceB(stBh, "h")
    xB_w, stBw = evacB(psB_w, "w")
    stsW = allreduceB(stBw, "w")
    A_Bh, B_Bh = bn_coeffs(stsH, 2, invNB, gB[:, 0:2], beB[:, 0:2], "Bsth")
    h2 = applyB(xB_h, A_Bh, B_Bh, "h")
    A_Bw, B_Bw = bn_coeffs(stsW, 2, invNB, gB[:, 2:4], beB[:, 2:4], "Bstw")
    w2 = applyB(xB_w, A_Bw, B_Bw, "w")

    # ======================= stage C convs (31 -> 32) =====================
    def stageC(wtile, src_chunks, b3t, name, out_dt):
        pss = convT(_W(wtile), [_A(s) for s in src_chunks], 31, 32, 2, 4, name,
                    widx=lambda ck, k, g: ((ck * 2 + k) * 4 + g) * 128)
        outs = []
        for g in range(4):
            h = sb.tile([128, BC * 32], out_dt, name=f"{name}3_{g}")
            for bh in range(2):
                ps = pss[(g, bh)]
                # psum cols t*16+j -> sbuf cols (bh*16+j)*32 + t
                inv = ps[:].rearrange("p (t j) -> p j t", j=16)
                dstv = h[:].rearrange("p (b t) -> p b t", t=32)[
                    :, bh * 16:(bh + 1) * 16, :]
                nc.scalar.activation(dstv, inv, Act.Tanh, bias=b3t[:, g:g + 1])
            outs.append(h)
        return outs

    h3 = stageC(wC_h, h2, b3h, "h", F32)
    GDTt = GDT
    w3 = stageC(wC_w, w2, b3w, "w", GDTt)

    # ======================= rank-weighted outer-product head =============
    # G[q][r, b*96 + ci*32 + hi] = c~[q][r, b*3+ci] * h3[q][r, b*32+hi]
    Gt = []
    for q in range(4):
        g = sb.tile([128, BC * 96], GDTt, name=f"G{q}")
        cv = cT[q][:].rearrange("p (b c) -> p b c", c=3).unsqueeze(3) \
            .broadcast_to([128, BC, 3, 32])
        hv = h3[q][:].rearrange("p (b h) -> p b h", h=32).unsqueeze(2) \
            .broadcast_to([128, BC, 3, 32])
        gv = g[:].rearrange("p (b c h) -> p b c h", c=3, h=32)
        eng = nc.gpsimd if q == 3 else nc.vector
        eng.tensor_tensor(gv, cv, hv, Alu.mult)
        Gt.append(g)

    outsb = sb.tile([96, BC * 32], F32, name="outsb")
    for grp in range(BC // 4):
        po = psp.tile([96, 4 * 32], F32, name=f"po{grp}", tag="ps")
        for j in range(4):
            bb = grp * 4 + j
            for q in range(4):
                nc.tensor.matmul(po[:, j * 32:(j + 1) * 32],
                                 Gt[q][:, bb * 96:(bb + 1) * 96],
                                 w3[q][:, bb * 32:(bb + 1) * 32],
                                 start=(j == 0 and q == 0),
                                 stop=(j == 3 and q == 3))
        nc.vector.tensor_copy(outsb[:, grp * 128:(grp + 1) * 128], po[:])

    # out[b, (ci,hi), wi] <- outsb[(ci,hi), b*32+wi]
    out_v = out_ap.rearrange("b c h w -> (c h) b w")
    nc.sync.dma_start(out_v, outsb[:].rearrange("p (b w) -> p b w", w=32))

    sb.release()
    psp.release()
    dram.release()


def _W(wtile):
    return _WView(wtile)


class _WView:
    """weight tile wrapper applying the f32r bitcast at slice time"""

    def __init__(self, t):
        self.t = t

    def __getitem__(self, key):
        return _mmbc(self.t[key])


class _AView:
    """activation tile wrapper applying the f32r bitcast at slice time"""

    def __init__(self, t):
        self.t = t

    def __getitem__(self, key):
        return _mmbc(self.t[key])


def _A(t):
    return _AView(t)


# --------------------------------------------------------------------------
# host side
# --------------------------------------------------------------------------

def _build_module():
    nc = bacc.Bacc("TRN2", target_bir_lowering=False, debug=False,
                   num_devices=N_CORES)
    WDT = _cdt()
    specs = {
        "onehot": ((NCLASS, B), F32),
        "lin_wt": ((NOISE, 128), F32), "emb_w": ((NCLASS, 128), F32),
        "noise_t": ((NOISE, B), F32),
        "bn0_g": ((128, 1), F32), "bn0_b": ((128, 1), F32),
        "gA": ((128, 6), F32), "beA": ((128, 6), F32),
        "coef_p": ((128, 4), F32), "gB": ((128, 4), F32), "beB": ((128, 4), F32),
        "b3h": ((128, 4), F32), "b3w": ((128, 4), F32),
        "wA_c": ((128, 3072), WDT), "wA_h": ((128, 4096), WDT),
        "wA_w": ((128, 4096), WDT),
        "wB_h": ((128, 4096), WDT), "wB_w": ((128, 4096), WDT),
        "wC_h": ((128, 2048), WDT), "wC_w": ((128, 2048), WDT),
    }
    I = {}
    for name, (shape, dt) in specs.items():
        I[name] = nc.dram_tensor(name, list(shape), dt, kind="ExternalInput").ap()
    out = nc.dram_tensor("out", [BC, 3, 32, 32], F32, kind="ExternalOutput")
    with tile.TileContext(nc) as tc:
        _emit(nc, tc, I, out.ap())
    nc.compile()
    return nc


def _np(x):
    return np.ascontiguousarray(np.asarray(x, dtype=np.float32))


def _pack_inputs(inputs):
    """host-side layout packing -> (replicated dict, per-core dicts)"""
    wnp = np.dtype(mybir.dt.np(_cdt()))
    noise = _np(inputs["noise"])
    label = np.asarray(inputs["label"]).astype(np.int64)

    c_w1 = _np(inputs["c_w1"])   # (256, 512, 3)
    h_w1 = _np(inputs["h_w1"])   # (256, 128, 16)
    w_w1 = _np(inputs["w_w1"])
    h_w2 = _np(inputs["h_w2"])   # (128, 256, 16)
    w_w2 = _np(inputs["w_w2"])
    h_w3 = _np(inputs["h_w3"])   # (256, 512, 2)
    w_w3 = _np(inputs["w_w3"])

    def packA_c(w):   # -> [ci_in, (chunk, k, g, co_in)]
        return np.ascontiguousarray(
            w.reshape(2, 128, 4, 128, 3).transpose(1, 0, 4, 2, 3).reshape(128, -1))

    def packA_h(w):   # (256,128,16) -> [ci_in, (chunk, k, co)]
        return np.ascontiguousarray(
            w.reshape(2, 128, 128, 16).transpose(1, 0, 3, 2).reshape(128, -1))

    def packB(w):     # (128,256,16) -> [ci, (k, g, co_in)]
        return np.ascontiguousarray(
            w.reshape(128, 2, 128, 16).transpose(0, 3, 1, 2).reshape(128, -1))

    def packC(w):     # (256,512,2) -> [ci_in, (chunk, k, g, co_in)]
        return np.ascontiguousarray(
            w.reshape(2, 128, 4, 128, 2).transpose(1, 0, 4, 2, 3).reshape(128, -1))

    def col128(*arrs):
        return np.ascontiguousarray(
            np.concatenate([a.reshape(-1, 128).T for a in arrs], axis=1))

    rep = {
        "lin_wt": _np(inputs["lin_w"]).T.copy(),
        "emb_w": _np(inputs["emb"]),
        "noise_t": noise.T.copy(),
        "bn0_g": _np(inputs["bn0_g"]).reshape(128, 1),
        "bn0_b": _np(inputs["bn0_b"]).reshape(128, 1),
        "gA": col128(_np(inputs["h_g1"]), _np(inputs["w_g1"]), _np(inputs["c_g1"])),
        "beA": col128(_np(inputs["h_be1"]), _np(inputs["w_be1"]), _np(inputs["c_be1"])),
        "coef_p": col128(_np(inputs["coef"])),
        "gB": col128(_np(inputs["h_g2"]), _np(inputs["w_g2"])),
        "beB": col128(_np(inputs["h_be2"]), _np(inputs["w_be2"])),
        "b3h": col128(_np(inputs["h_b3"])),
        "b3w": col128(_np(inputs["w_b3"])),
        "wA_c": packA_c(c_w1).astype(wnp),
        "wA_h": packA_h(h_w1).astype(wnp),
        "wA_w": packA_h(w_w1).astype(wnp),
        "wB_h": packB(h_w2).astype(wnp),
        "wB_w": packB(w_w2).astype(wnp),
        "wC_h": packC(h_w3).astype(wnp),
        "wC_w": packC(w_w3).astype(wnp),
    }

    noise_t = rep.pop("noise_t")
    per_core = []
    for c in range(N_CORES):
        own = np.arange(c * BC, (c + 1) * BC)
        rest = np.delete(np.arange(B), own)
        perm = np.concatenate([own, rest])
        oh = (label[perm][None, :] == np.arange(NCLASS)[:, None]).astype(np.float32)
        per_core.append({
            "noise_t": np.ascontiguousarray(noise_t[:, perm]),
            "onehot": np.ascontiguousarray(oh),
            **rep,
        })
    return per_core


def kernel(**inputs) -> np.ndarray:
    with _lock:
        key = (PREC, NO_AR, SIM_SAFE)
        nc = _cache.get(key)
        if nc is None:
            nc = _build_module()
            _cache[key] = nc
    in_maps = _pack_inputs(inputs)
    res = run_bass_kernel_spmd(nc, in_maps, core_ids=list(range(N_CORES)))
    return np.concatenate([r["out"] for r in res.results], axis=0)


# revision 24
# speedup vs baseline: 1.0386x; 1.0386x over previous
"""Trainium2 Bass kernel for nn_DecompGen (conditional generator + rank-decomposed
outer-product head).

Sharding: pure data parallelism — batch B=256 is split 32-per-core across 8
NeuronCores; the small parameter set is replicated.  Training-mode BatchNorm
needs full-batch statistics, so the kernel does two tiny AllReduces (one for
the stage-A convs' stats, one for stage-B).  The first BN (on the input
linear) is handled by replicating that stage's trivially small matmul on
every core, which removes a third AllReduce.

Layout conventions (per core, channels on SBUF partitions):
  - conv activations are stored "t-major": tile column = t*32 + b
  - convT is computed as K shifted matmuls accumulating into one PSUM bank;
    each weight-tap k contributes to output window t in [k, k+L_in); windows
    are emitted as (covered + fresh) split matmuls so per-element PSUM
    has_written semantics stay uniform per instruction.
  - the rank-weighted outer-product head runs per-sample matmuls
    out[(ci,hi), wi] = sum_r (coef*c)[r,ci]*h[r,hi] . w[r,wi]
"""

import os
import threading

import numpy as np

import concourse.bacc as bacc
import concourse.mybir as mybir
import concourse.tile as tile
from concourse.bass_utils import run_bass_kernel_spmd

F32 = mybir.dt.float32
F32R = mybir.dt.float32r
BF16 = mybir.dt.bfloat16

N_CORES = 8
B, NOISE, NCLASS, RANK = 256, 100, 10, 512
R4, R2 = RANK // 4, RANK // 2  # 128, 256
BC = B // N_CORES  # 32 samples per core
EPS = 1e-5

# precision config: "f32" (exact), "f32r" (fp32 storage, fast PE mode on the
# big convs), "wbf16" (bf16 weights + f32r activations), "bf16" (bf16
# weights/activations on the conv + einsum path)
PREC = os.environ.get("KERNEL_PREC", "f32r")
NO_AR = os.environ.get("KERNEL_NO_AR", "0") == "1"
SIM_SAFE = os.environ.get("KERNEL_SIM_SAFE", "0") == "1"

_lock = threading.Lock()
_cache: dict = {}


def _cdt():
    """storage dtype of conv WEIGHTS"""
    if PREC in ("bf16", "wbf16"):
        return BF16
    if PREC == "f32r":
        return F32R
    return F32


def _adt():
    """storage dtype of conv ACTIVATIONS (matmul moving operands)"""
    if PREC == "bf16":
        return BF16
    if PREC in ("f32r", "wbf16"):
        return F32R
    return F32


def _mmbc(ap):
    return ap


# --------------------------------------------------------------------------
# device kernel
# --------------------------------------------------------------------------

def _emit(nc, tc, I, out_ap):
    ADT = _adt()          # conv activations (h1/h2, latent)
    WDT = _cdt()          # conv weights (host already cast)
    # einsum G / w3 dtype: the head matmuls are weight-load bound either
    # way (M=96 < 128 disables FWL), so fp32 costs ~nothing and removes the
    # dominant bf16 rounding term from the output.  bf16 only in bf16 mode.
    GDT = BF16 if PREC == "bf16" else F32
    Alu = mybir.AluOpType
    Act = mybir.ActivationFunctionType

    sb = tc.alloc_tile_pool(name="sb", bufs=1)
    # all PSUM tiles are <= one bank; share 8 rotating bank slots
    psp = tc.alloc_tile_pool(name="ps", bufs=8, space="PSUM")
    dram = tc.alloc_tile_pool(name="dram", bufs=1, space="DRAM")

    def load(name, shape, dtype=F32, eng=None, chunks=1):
        t = sb.tile(list(shape), dtype, name=f"sb_{name}")
        eng = eng or nc.sync
        n = shape[1]
        step = (n + chunks - 1) // chunks
        for c0 in range(0, n, step):
            c1 = min(n, c0 + step)
            eng.dma_start(t[:, c0:c1], I[name][:, c0:c1])
        return t

    # ---- input loads: small tensors ride in 3 packed arrays (one DMA each
    # instead of ~13 — every HWDGE dma pays ~1us of queue latency) ----
    p100 = load("p100", (NOISE, 128 + B))
    lin_wt = p100[:, 0:128]
    noise_t = p100[:, 128:128 + B]
    p10 = load("p10", (NCLASS, B + 128))
    onehot = p10[:, 0:B]
    emb_w = p10[:, B:B + 128]
    p128 = load("p128", (128, 34))
    bn0_g = p128[:, 0:1]
    bn0_b = p128[:, 1:2]
    gA = p128[:, 2:8]
    beA = p128[:, 8:14]
    coef_p = p128[:, 14:18]
    gB = p128[:, 18:22]
    beB = p128[:, 22:26]
    b3h = p128[:, 26:30]
    b3w = p128[:, 30:34]
    wA_h = load("wA_h", (128, 2 * 16 * 128), WDT, chunks=4)
    wA_w = load("wA_w", (128, 2 * 16 * 128), WDT, eng=nc.scalar, chunks=4)
    wA_c = load("wA_c", (128, 2 * 3 * 4 * 128), WDT, eng=nc.gpsimd, chunks=2)
    wB_h = load("wB_h", (128, 16 * 2 * 128), WDT, chunks=4)
    wB_w = load("wB_w", (128, 16 * 2 * 128), WDT, eng=nc.scalar, chunks=4)
    wC_h = load("wC_h", (128, 2 * 2 * 4 * 128), WDT, eng=nc.gpsimd, chunks=2)
    wC_w = load("wC_w", (128, 2 * 2 * 4 * 128), WDT, eng=nc.scalar, chunks=2)

    # DMA priority: stage-A h weights, then conv2-h weights, then the rest
    for t in (wA_h, wB_h, wA_w, wB_w, wA_c, wC_h, wC_w):
        pass

    eps_c = sb.tile([128, 1], F32, name="eps_c")
    nc.vector.memset(eps_c[:], EPS)
    invNB = sb.tile([128, 4], F32, name="invNB")
    nc.vector.memset(invNB[:], 1.0 / (B * 31))

    # ============== stage 0 + stage A (replicated, full batch) ============
    # Every core computes the full-batch pre-BN activations of the cheap
    # early stages, so all their BatchNorm statistics are local — no
    # AllReduce needed until stage B.  The host permutes the batch per core
    # (own 32 samples first), so "my shard" is always columns 0:32.
    ps_zf = psp.tile([128, B], F32, name="ps_zf", tag="ps")
    nc.tensor.matmul(ps_zf[:], lin_wt, noise_t, start=True, stop=True)
    ps_lab = psp.tile([128, B], F32, name="ps_lab", tag="ps")
    nc.tensor.matmul(ps_lab[:], emb_w, onehot, start=True, stop=True)

    st0 = sb.tile([128, 6], F32, name="st0")
    nc.vector.bn_stats(st0[:], ps_zf[:])
    mv0 = sb.tile([128, 2], F32, name="mv0")
    nc.vector.bn_aggr(mv0[:], st0[:])
    std0 = sb.tile([128, 1], F32, name="std0")
    nc.scalar.activation(std0[:], mv0[:, 1:2], Act.Sqrt, bias=eps_c[:])
    rstd0 = sb.tile([128, 1], F32, name="rstd0")
    nc.vector.reciprocal(rstd0[:], std0[:])
    A0 = sb.tile([128, 1], F32, name="A0")
    nc.vector.tensor_tensor(A0[:], rstd0[:], bn0_g, Alu.mult)
    t0_ = sb.tile([128, 1], F32, name="t0_")
    nc.vector.tensor_tensor(t0_[:], mv0[:, 0:1], A0[:], Alu.mult)
    B0 = sb.tile([128, 1], F32, name="B0")
    nc.vector.tensor_tensor(B0[:], bn0_b, t0_[:], Alu.subtract)

    # latent = [lrelu(bn0(z)) ; emb[label]]  (2 chunks of 128 ch, full batch)
    lat0 = sb.tile([128, B], ADT, name="lat0")
    nc.scalar.activation(lat0[:], ps_zf[:], Act.Identity, bias=B0[:], scale=A0[:])
    nc.vector.scalar_tensor_tensor(lat0[:], lat0[:], 0.01, lat0[:], Alu.mult, Alu.max)
    lat1 = sb.tile([128, B], ADT, name="lat1")
    nc.vector.tensor_copy(lat1[:], ps_lab[:])
    lat = [lat0, lat1]

    # ---- stage A convs (L_in=1): x[co, t, b] = sum_ci w[ci,co,t]*lat[ci,b]
    # full batch per k-block in PSUM; bn_stats per block (equal sizes) then
    # bn_aggr -> local full-batch (mean, var); own shard evacuated to SBUF.
    mvA = sb.tile([128, 12], F32, name="mvA")

    def stageA_conv(wtile, KA, G, kc, mvcol, name):
        outs = []
        for g in range(G):
            nblk = KA // kc
            stS = sb.tile([128, 48], F32, name=f"stA_{name}{g}", tag="stS", bufs=2)
            xdt = F32 if (ADT == BF16 or G > 1) else ADT
            x = sb.tile([128, KA * BC], xdt, name=f"xA_{name}{g}")
            for blk in range(nblk):
                ps = psp.tile([128, kc * B], F32, name=f"psA_{name}{g}_{blk}",
                              tag="ps")
                i = 0
                for kk in range(kc):
                    k = blk * kc + kk
                    for chunk in range(2):
                        idx = ((k * 2 + chunk) * G + g) * 128
                        nc.tensor.matmul(ps[:, kk * B:(kk + 1) * B],
                                         wtile[:, idx:idx + 128], lat[chunk][:],
                                         start=(i == 0), stop=(i == 2 * kc - 1))
                        i += 1
                nc.vector.bn_stats(stS[:, blk * 6:(blk + 1) * 6], ps[:])
                src = ps[:].rearrange("p (kk b) -> p kk b", b=B)[:, :, 0:BC]
                dst = x[:].rearrange("p (t b) -> p t b", b=BC)[
                    :, blk * kc:(blk + 1) * kc, :]
                nc.scalar.copy(dst, src)
            nc.vector.bn_aggr(mvA[:, (mvcol + g) * 2:(mvcol + g) * 2 + 2], stS[:, 0:nblk * 6])
            outs.append(x)
        return outs

    xA_h = stageA_conv(wA_h, 16, 1, 2, 0, "h")[0]
    xA_w = stageA_conv(wA_w, 16, 1, 2, 1, "w")[0]
    xA_c = stageA_conv(wA_c, 3, 4, 1, 2, "c")

    # ---- per-channel BN coefficients: A = g/sqrt(var+eps), B = be - mean*A
    def bn_from_mv(mean_v, var_v, ncols, g_t, be_t, name):
        std = sb.tile([128, ncols], F32, name=f"std{name}")
        nc.scalar.activation(std[:], var_v, Act.Sqrt, bias=eps_c[:])
        rstd = sb.tile([128, ncols], F32, name=f"rstd{name}")
        nc.vector.reciprocal(rstd[:], std[:])
        Atl = sb.tile([128, ncols], F32, name=f"A{name}")
        nc.vector.tensor_tensor(Atl[:], rstd[:], g_t, Alu.mult)
        tmp = sb.tile([128, ncols], F32, name=f"tmp{name}")
        nc.vector.tensor_tensor(tmp[:], mean_v, Atl[:], Alu.mult)
        Btl = sb.tile([128, ncols], F32, name=f"B{name}")
        nc.vector.tensor_tensor(Btl[:], be_t, tmp[:], Alu.subtract)
        return Atl, Btl

    def bn_coeffs(stg, ncols, invN, g_t, be_t, name):
        sc = sb.tile([128, 2 * ncols], F32, name=f"sc{name}")
        nc.vector.tensor_tensor(sc[:], stg[:], invN[:], Alu.mult)
        mean = sc[:, 0:ncols]
        ex2 = sc[:, ncols:2 * ncols]
        msq = sb.tile([128, ncols], F32, name=f"msq{name}")
        nc.scalar.activation(msq[:], mean, Act.Square)
        var = sb.tile([128, ncols], F32, name=f"var{name}")
        nc.vector.tensor_tensor(var[:], ex2, msq[:], Alu.subtract)
        return bn_from_mv(mean, var[:], ncols, g_t, be_t, name)

    # independent per-path coefficient chains so each conv path unblocks as
    # soon as its own stats are in
    A_Ah, B_Ah = bn_from_mv(mvA[:, 0:1], mvA[:, 1:2], 1, gA[:, 0:1], beA[:, 0:1], "Ah")
    A_Aw, B_Aw = bn_from_mv(mvA[:, 2:3], mvA[:, 3:4], 1, gA[:, 1:2], beA[:, 1:2], "Aw")
    mvc = mvA[:].rearrange("p (c two) -> p c two", two=2)[:, 2:6, :]
    A_Ac, B_Ac = bn_from_mv(mvc[:, :, 0:1].squeeze(2), mvc[:, :, 1:2].squeeze(2),
                            4, gA[:, 2:6], beA[:, 2:6], "Ac")
    # fold the rank weights into the c-path coefficients (coef >= 0 commutes
    # with leaky-relu)
    nc.vector.tensor_tensor(A_Ac[:], A_Ac[:], coef_p, Alu.mult)
    nc.vector.tensor_tensor(B_Ac[:], B_Ac[:], coef_p, Alu.mult)

    def bn_apply(dst, src, Atl, Btl, col, slope):
        nc.scalar.activation(dst, src, Act.Identity,
                             bias=Btl[:, col:col + 1], scale=Atl[:, col:col + 1])
        nc.vector.scalar_tensor_tensor(dst, dst, slope, dst, Alu.mult, Alu.max)

    if ADT != BF16:
        h1, w1 = xA_h, xA_w
        bn_apply(h1[:], h1[:], A_Ah, B_Ah, 0, 0.2)
        bn_apply(w1[:], w1[:], A_Aw, B_Aw, 0, 0.2)
    else:
        h1 = sb.tile([128, 16 * BC], ADT, name="h1")
        bn_apply(h1[:], xA_h[:], A_Ah, B_Ah, 0, 0.2)
        w1 = sb.tile([128, 16 * BC], ADT, name="w1")
        bn_apply(w1[:], xA_w[:], A_Aw, B_Aw, 0, 0.2)
    # c~ = coef * lrelu(bn(c_pre)): written b-major (cols b*3+ci) for the head
    cT = []
    for g in range(4):
        ct = sb.tile([128, 3 * BC], F32, name=f"cT{g}")
        src = xA_c[g][:].rearrange("p (c b) -> p c b", b=BC)
        dstv = ct[:].rearrange("p (b c) -> p c b", c=3)
        bn_apply(dstv, src, A_Ac, B_Ac, g, 0.2)
        cT.append(ct)

    # ======================= stage B convs (16 -> 31) =====================
    # y[co, t, b] += sum_ci w2[ci, co, k] * h1[ci, t-k, b]

    def convT(wtile, src_chunks, LI, LO, KK, G, psname, n_bh=2, widx=None):
        """shifted-window convT: returns psum tiles [(g, bh) -> [128, LO*16]]

        Default emits one full-window matmul per (k, chunk) — on HW the
        per-element has_written bits make partially-overlapping windows
        accumulate correctly.  CoreSim asserts uniform pending state per
        matmul, so SIM_SAFE mode splits each k>=1 window into an
        all-covered part plus a single fresh output column.
        """
        pss = {}
        n_ck = len(src_chunks)
        for g in range(G):
            for bh in range(n_bh):
                ps = psp.tile([128, LO * 16], F32, name=f"ps{psname}_{g}_{bh}",
                              tag="ps")
                i = 0
                for k in range(KK):
                    for ck in range(n_ck):
                        lw = wtile[:, widx(ck, k, g):widx(ck, k, g) + 128]
                        src = src_chunks[ck]
                        last = (k == KK - 1 and ck == n_ck - 1)
                        if k == 0 or not SIM_SAFE:
                            rhs = src[:].rearrange("p (t b) -> p t b", b=BC)[
                                :, 0:LI, bh * 16:(bh + 1) * 16]
                            nc.tensor.matmul(ps[:, k * 16:(k + LI) * 16], lw, rhs,
                                             start=(i == 0), stop=last)
                            i += 1
                        else:
                            rhs = src[:].rearrange("p (t b) -> p t b", b=BC)[
                                :, 0:LI - 1, bh * 16:(bh + 1) * 16]
                            nc.tensor.matmul(
                                ps[:, k * 16:(k + LI - 1) * 16], lw, rhs,
                                start=False, stop=False)
                            i += 1
                            rhs2 = src[:, (LI - 1) * BC + bh * 16:
                                       (LI - 1) * BC + bh * 16 + 16]
                            nc.tensor.matmul(
                                ps[:, (k + LI - 1) * 16:(k + LI) * 16], lw, rhs2,
                                start=False, stop=last)
                            i += 1
                pss[(g, bh)] = ps
        return pss

    psB_h = convT(_W(wB_h), [_A(h1)], 16, 31, 16, 2, "Bh",
                  widx=lambda ck, k, g: (k * 2 + g) * 128)
    psB_w = convT(_W(wB_w), [_A(w1)], 16, 31, 16, 2, "Bw",
                  widx=lambda ck, k, g: (k * 2 + g) * 128)

    # evacuate + stats; xB tiles are [128, 31*BC] t-major over full b.
    # Each path (h, w) gets its own small AllReduce so the h-path collective
    # overlaps the w-path convs (and vice versa for stage C).
    def evacB(pss, name):
        stT = sb.tile([128, 8], F32, name=f"stB{name}")
        outs = []
        for g in range(2):
            xdt = F32 if ADT == BF16 else ADT
            x = sb.tile([128, 31 * BC], xdt, name=f"xB_{name}{g}")
            for bh in range(2):
                ps = pss[(g, bh)]
                col = g * 2 + bh
                dstv = x[:].rearrange("p (t b) -> p t b", b=BC)[:, :, bh * 16:(bh + 1) * 16]
                nc.vector.tensor_scalar(dstv, ps[:], 1.0, None, Alu.mult, Alu.add,
                                        accum_out=stT[:, col:col + 1])
                scr = sb.tile([128, 512], F32, name=f"scrB_{name}{g}{bh}",
                              tag="scr", bufs=1)
                nc.scalar.activation(scr[:, 0:31 * 16], ps[:], Act.Square,
                                     accum_out=stT[:, 4 + col:5 + col])
            outs.append(x)
        return outs, stT

    def allreduceB(stT, name):
        ar_in = dram.tile([128, 8], F32, name=f"arB{name}_in")
        ar_out = dram.tile([128, 8], F32, name=f"arB{name}_out")
        nc.sync.dma_start(ar_in[:], stT[:])
        if NO_AR:
            nc.gpsimd.dma_start(ar_out[:], ar_in[:])
        else:
            nc.gpsimd.collective_compute("AllReduce", Alu.add,
                                         replica_groups=[list(range(N_CORES))],
                                         ins=[ar_in.opt()], outs=[ar_out.opt()])
        stg = sb.tile([128, 8], F32, name=f"stBg{name}")
        nc.sync.dma_start(stg[:], ar_out[:])
        # combine the two batch-half partial sums: cols (s1 g0, s1 g1, s2 g0, s2 g1)
        sts = sb.tile([128, 4], F32, name=f"stBs{name}")
        v = stg[:].rearrange("p (c two) -> p c two", two=2)
        nc.vector.tensor_tensor(sts[:], v[:, :, 0:1].squeeze(2),
                                v[:, :, 1:2].squeeze(2), Alu.add)
        return sts

    def applyB(xs, A_t, B_t, name):
        outs = []
        for g in range(2):
            if ADT != BF16:
                t = xs[g]
                bn_apply(t[:], t[:], A_t, B_t, g, 0.2)
            else:
                t = sb.tile([128, 31 * BC], ADT, name=f"{name}2_{g}")
                bn_apply(t[:], xs[g][:], A_t, B_t, g, 0.2)
            outs.append(t)
        return outs

    xB_h, stBh = evacB(psB_h, "h")
    stsH = allreduceB(stBh, "h")
    xB_w, stBw = evacB(psB_w, "w")
    stsW = allreduceB(stBw, "w")
    A_Bh, B_Bh = bn_coeffs(stsH, 2, invNB, gB[:, 0:2], beB[:, 0:2], "Bsth")
    h2 = applyB(xB_h, A_Bh, B_Bh, "h")
    A_Bw, B_Bw = bn_coeffs(stsW, 2, invNB, gB[:, 2:4], beB[:, 2:4], "Bstw")
    w2 = applyB(xB_w, A_Bw, B_Bw, "w")

    # ======================= stage C convs (31 -> 32) =====================
    def stageC(wtile, src_chunks, b3t, name, out_dt):
        pss = convT(_W(wtile), [_A(s) for s in src_chunks], 31, 32, 2, 4, name,
                    widx=lambda ck, k, g: ((k * 2 + ck) * 4 + g) * 128)
        outs = []
        for g in range(4):
            h = sb.tile([128, BC * 32], out_dt, name=f"{name}3_{g}")
            for bh in range(2):
                ps = pss[(g, bh)]
                # psum cols t*16+j -> sbuf cols (bh*16+j)*32 + t
                inv = ps[:].rearrange("p (t j) -> p j t", j=16)
                dstv = h[:].rearrange("p (b t) -> p b t", t=32)[
                    :, bh * 16:(bh + 1) * 16, :]
                nc.scalar.activation(dstv, inv, Act.Tanh, bias=b3t[:, g:g + 1])
            outs.append(h)
        return outs

    h3 = stageC(wC_h, h2, b3h, "h", F32)
    GDTt = GDT
    w3 = stageC(wC_w, w2, b3w, "w", GDTt)

    # ======================= rank-weighted outer-product head =============
    # G[q][r, b*96 + ci*32 + hi] = c~[q][r, b*3+ci] * h3[q][r, b*32+hi]
    Gt = []
    for q in range(4):
        g = sb.tile([128, BC * 96], GDTt, name=f"G{q}")
        cv = cT[q][:].rearrange("p (b c) -> p b c", c=3).unsqueeze(3) \
            .broadcast_to([128, BC, 3, 32])
        hv = h3[q][:].rearrange("p (b h) -> p b h", h=32).unsqueeze(2) \
            .broadcast_to([128, BC, 3, 32])
        gv = g[:].rearrange("p (b c h) -> p b c h", c=3, h=32)
        eng = nc.gpsimd if q == 3 else nc.vector
        eng.tensor_tensor(gv, cv, hv, Alu.mult)
        Gt.append(g)

    outsb = sb.tile([96, BC * 32], F32, name="outsb")
    out_v = out_ap.rearrange("b c h w -> (c h) b w")
    for grp in range(BC // 4):
        po = psp.tile([96, 4 * 32], F32, name=f"po{grp}", tag="ps")
        for q in range(4):
            for j in range(4):
                bb = grp * 4 + j
                nc.tensor.matmul(po[:, j * 32:(j + 1) * 32],
                                 Gt[q][:, bb * 96:(bb + 1) * 96],
                                 w3[q][:, bb * 32:(bb + 1) * 32],
                                 start=(j == 0 and q == 0),
                                 stop=(j == 3 and q == 3))
        nc.vector.tensor_copy(outsb[:, grp * 128:(grp + 1) * 128], po[:])
        if grp % 4 == 3:
            # stream each 16-sample half out as soon as it is evacuated
            h0 = (grp - 3) * 4
            nc.sync.dma_start(
                out_v[:, h0:h0 + 16, :],
                outsb[:, h0 * 32:(h0 + 16) * 32].rearrange("p (b w) -> p b w", w=32))

    sb.release()
    psp.release()
    dram.release()


def _W(wtile):
    return _WView(wtile)


class _WView:
    """weight tile wrapper applying the f32r bitcast at slice time"""

    def __init__(self, t):
        self.t = t

    def __getitem__(self, key):
        return _mmbc(self.t[key])


class _AView:
    """activation tile wrapper applying the f32r bitcast at slice time"""

    def __init__(self, t):
        self.t = t

    def __getitem__(self, key):
        return _mmbc(self.t[key])


def _A(t):
    return _AView(t)


# --------------------------------------------------------------------------
# host side
# --------------------------------------------------------------------------

def _build_module():
    nc = bacc.Bacc("TRN2", target_bir_lowering=False, debug=False,
                   num_devices=N_CORES)
    WDT = _cdt()
    specs = {
        "p100": ((NOISE, 128 + B), F32),
        "p10": ((NCLASS, B + 128), F32),
        "p128": ((128, 34), F32),
        "wA_c": ((128, 3072), WDT), "wA_h": ((128, 4096), WDT),
        "wA_w": ((128, 4096), WDT),
        "wB_h": ((128, 4096), WDT), "wB_w": ((128, 4096), WDT),
        "wC_h": ((128, 2048), WDT), "wC_w": ((128, 2048), WDT),
    }
    I = {}
    for name, (shape, dt) in specs.items():
        I[name] = nc.dram_tensor(name, list(shape), dt, kind="ExternalInput").ap()
    out = nc.dram_tensor("out", [BC, 3, 32, 32], F32, kind="ExternalOutput")
    with tile.TileContext(nc) as tc:
        _emit(nc, tc, I, out.ap())
    nc.compile()
    return nc


def _np(x):
    return np.ascontiguousarray(np.asarray(x, dtype=np.float32))


def _pack_inputs(inputs):
    """host-side layout packing -> (replicated dict, per-core dicts)"""
    wnp = np.dtype(mybir.dt.np(_cdt()))
    noise = _np(inputs["noise"])
    label = np.asarray(inputs["label"]).astype(np.int64)

    c_w1 = _np(inputs["c_w1"])   # (256, 512, 3)
    h_w1 = _np(inputs["h_w1"])   # (256, 128, 16)
    w_w1 = _np(inputs["w_w1"])
    h_w2 = _np(inputs["h_w2"])   # (128, 256, 16)
    w_w2 = _np(inputs["w_w2"])
    h_w3 = _np(inputs["h_w3"])   # (256, 512, 2)
    w_w3 = _np(inputs["w_w3"])

    def packA_c(w):   # -> [ci_in, (k, chunk, g, co_in)]
        return np.ascontiguousarray(
            w.reshape(2, 128, 4, 128, 3).transpose(1, 4, 0, 2, 3).reshape(128, -1))

    def packA_h(w):   # (256,128,16) -> [ci_in, (k, chunk, co)]
        return np.ascontiguousarray(
            w.reshape(2, 128, 128, 16).transpose(1, 3, 0, 2).reshape(128, -1))

    def packB(w):     # (128,256,16) -> [ci, (k, g, co_in)]
        return np.ascontiguousarray(
            w.reshape(128, 2, 128, 16).transpose(0, 3, 1, 2).reshape(128, -1))

    def packC(w):     # (256,512,2) -> [ci_in, (k, chunk, g, co_in)]
        return np.ascontiguousarray(
            w.reshape(2, 128, 4, 128, 2).transpose(1, 4, 0, 2, 3).reshape(128, -1))

    def col128(*arrs):
        return np.ascontiguousarray(
            np.concatenate([a.reshape(-1, 128).T for a in arrs], axis=1))

    p128 = np.concatenate([
        _np(inputs["bn0_g"]).reshape(128, 1),
        _np(inputs["bn0_b"]).reshape(128, 1),
        col128(_np(inputs["h_g1"]), _np(inputs["w_g1"]), _np(inputs["c_g1"])),
        col128(_np(inputs["h_be1"]), _np(inputs["w_be1"]), _np(inputs["c_be1"])),
        col128(_np(inputs["coef"])),
        col128(_np(inputs["h_g2"]), _np(inputs["w_g2"])),
        col128(_np(inputs["h_be2"]), _np(inputs["w_be2"])),
        col128(_np(inputs["h_b3"])),
        col128(_np(inputs["w_b3"])),
    ], axis=1)

    rep = {
        "p128": np.ascontiguousarray(p128),
        "wA_c": packA_c(c_w1).astype(wnp),
        "wA_h": packA_h(h_w1).astype(wnp),
        "wA_w": packA_h(w_w1).astype(wnp),
        "wB_h": packB(h_w2).astype(wnp),
        "wB_w": packB(w_w2).astype(wnp),
        "wC_h": packC(h_w3).astype(wnp),
        "wC_w": packC(w_w3).astype(wnp),
    }

    lin_wt = _np(inputs["lin_w"]).T
    emb_w = _np(inputs["emb"])
    noise_t = noise.T
    per_core = []
    for c in range(N_CORES):
        own = np.arange(c * BC, (c + 1) * BC)
        rest = np.delete(np.arange(B), own)
        perm = np.concatenate([own, rest])
        oh = (label[perm][None, :] == np.arange(NCLASS)[:, None]).astype(np.float32)
        per_core.append({
            "p100": np.ascontiguousarray(
                np.concatenate([lin_wt, noise_t[:, perm]], axis=1)),
            "p10": np.ascontiguousarray(np.concatenate([oh, emb_w], axis=1)),
            **rep,
        })
    return per_core


def kernel(**inputs) -> np.ndarray:
    with _lock:
        key = (PREC, NO_AR, SIM_SAFE)
        nc = _cache.get(key)
        if nc is None:
            nc = _build_module()
            _cache[key] = nc
    in_maps = _pack_inputs(inputs)
    res = run_bass_kernel_spmd(nc, in_maps, core_ids=list(range(N_CORES)))
    return np.concatenate([r["out"] for r in res.results], axis=0)


# revision 29
# speedup vs baseline: 19662.7765x; 18932.7303x over previous
"""Trainium2 Bass kernel for nn_DecompGen (conditional generator + rank-decomposed
outer-product head).

Sharding: data parallelism — batch B=256 is split 32-per-core across 8
NeuronCores; the small parameter set is replicated.

Training-mode BatchNorm needs full-batch statistics.  The cheap early stages
(input linear + the three L_in=1 convs) are therefore computed REPLICATED at
full batch on every core — their BN stats are then local (bn_stats/bn_aggr),
and the host permutes the batch per core (own 32 samples first) so each
core's shard is always columns 0:32 with no runtime indexing.  Only the two
expensive stage-B convs run sharded; their per-channel sum/sum-of-squares
stats take one tiny AllReduce per path (h, w), each overlapping the other
path's compute.

Layout conventions (per core, channels on SBUF partitions):
  - conv activations are "t-major": tile column = t*32 + b
  - convT accumulates K shifted matmuls into one PSUM bank; weight-tap k
    covers output window t in [k, k+L_in).  On HW the per-element PSUM
    has_written bits make overlapping windows accumulate correctly; CoreSim
    requires uniform pending state per matmul, so KERNEL_SIM_SAFE=1 splits
    each k>=1 window into an all-covered part plus the single fresh column.
  - float32r (fp32 storage, PE fast path) is used for the conv chain; the
    output head and all BN math stay fp32.  Measured absmax-relative error
    vs the fp32 reference: ~5.6e-4.
  - the rank-weighted outer-product head runs per-sample matmuls
    out[(ci,hi), wi] = sum_r (coef*c)[r,ci]*h[r,hi] . w[r,wi]
"""

import os
import threading

import numpy as np

import concourse.bacc as bacc
import concourse.mybir as mybir
import concourse.tile as tile
from concourse.bass_utils import run_bass_kernel_spmd

F32 = mybir.dt.float32
F32R = mybir.dt.float32r
BF16 = mybir.dt.bfloat16

N_CORES = 8
B, NOISE, NCLASS, RANK = 256, 100, 10, 512
R4, R2 = RANK // 4, RANK // 2  # 128, 256
BC = B // N_CORES  # 32 samples per core
EPS = 1e-5

# precision config: "f32" (exact), "f32r" (fp32 storage, fast PE mode on the
# big convs), "wbf16" (bf16 weights + f32r activations), "bf16" (bf16
# weights/activations on the conv + einsum path)
PREC = os.environ.get("KERNEL_PREC", "f32r")
NO_AR = os.environ.get("KERNEL_NO_AR", "0") == "1"
SIM_SAFE = os.environ.get("KERNEL_SIM_SAFE", "0") == "1"

_lock = threading.Lock()
_cache: dict = {}


def _cdt():
    """storage dtype of conv WEIGHTS"""
    if PREC in ("bf16", "wbf16"):
        return BF16
    if PREC == "f32r":
        return F32R
    return F32


def _adt():
    """storage dtype of conv ACTIVATIONS (matmul moving operands)"""
    if PREC == "bf16":
        return BF16
    if PREC in ("f32r", "wbf16"):
        return F32R
    return F32


# --------------------------------------------------------------------------
# device kernel
# --------------------------------------------------------------------------

def _emit(nc, tc, I, out_ap):
    ADT = _adt()          # conv activations (h1/h2, latent)
    WDT = _cdt()          # conv weights (host already cast)
    # einsum G / w3 dtype: the head matmuls are weight-load bound either
    # way (M=96 < 128 disables FWL), so fp32 costs ~nothing and removes the
    # dominant bf16 rounding term from the output.
    GDT = F32
    Alu = mybir.AluOpType
    Act = mybir.ActivationFunctionType

    sb = tc.alloc_tile_pool(name="sb", bufs=1)
    # all PSUM tiles are <= one bank; share 8 rotating bank slots
    psp = tc.alloc_tile_pool(name="ps", bufs=8, space="PSUM")
    dram = tc.alloc_tile_pool(name="dram", bufs=1, space="DRAM")

    def load(name, shape, dtype=F32, eng=None, chunks=1):
        t = sb.tile(list(shape), dtype, name=f"sb_{name}")
        eng = eng or nc.sync
        n = shape[1]
        step = (n + chunks - 1) // chunks
        for c0 in range(0, n, step):
            c1 = min(n, c0 + step)
            eng.dma_start(t[:, c0:c1], I[name][:, c0:c1])
        return t

    # ---- input loads: small tensors ride in 3 packed arrays (one DMA each
    # instead of ~13 — every HWDGE dma pays ~1us of queue latency) ----
    p100 = load("p100", (NOISE, 128 + B))
    lin_wt = p100[:, 0:128]
    noise_t = p100[:, 128:128 + B]
    p10 = load("p10", (NCLASS, B + 128))
    onehot = p10[:, 0:B]
    emb_w = p10[:, B:B + 128]
    p128 = load("p128", (128, 34))
    bn0_g = p128[:, 0:1]
    bn0_b = p128[:, 1:2]
    gA = p128[:, 2:8]
    beA = p128[:, 8:14]
    coef_p = p128[:, 14:18]
    gB = p128[:, 18:22]
    beB = p128[:, 22:26]
    b3h = p128[:, 26:30]
    b3w = p128[:, 30:34]
    # stage A+B weights: h/w chunks interleaved on one queue so both conv
    # paths stream in at the same rate (PE consumes them alternately)
    wA_h = sb.tile([128, 2 * 16 * 128], WDT, name="sb_wA_h")
    wA_w = sb.tile([128, 2 * 16 * 128], WDT, name="sb_wA_w")
    wB_h = sb.tile([128, 16 * 2 * 128], WDT, name="sb_wB_h")
    wB_w = sb.tile([128, 16 * 2 * 128], WDT, name="sb_wB_w")
    for c0 in range(0, 4096, 1024):
        nc.sync.dma_start(wA_h[:, c0:c0 + 1024], I["wA_h"][:, c0:c0 + 1024])
        nc.sync.dma_start(wA_w[:, c0:c0 + 1024], I["wA_w"][:, c0:c0 + 1024])
    for c0 in range(0, 4096, 1024):
        nc.sync.dma_start(wB_h[:, c0:c0 + 1024], I["wB_h"][:, c0:c0 + 1024])
        nc.sync.dma_start(wB_w[:, c0:c0 + 1024], I["wB_w"][:, c0:c0 + 1024])
    # c path feeds the AllReduce-window filler work (~60us in); conv3 weights
    # are needed a bit later
    wA_c = load("wA_c", (128, 2 * 3 * 4 * 128), WDT, eng=nc.gpsimd, chunks=2)
    wC_h = load("wC_h", (128, 2 * 2 * 4 * 128), WDT, eng=nc.gpsimd, chunks=2)
    wC_w = load("wC_w", (128, 2 * 2 * 4 * 128), WDT, eng=nc.scalar, chunks=2)

    eps_c = sb.tile([128, 1], F32, name="eps_c")
    nc.vector.memset(eps_c[:], EPS)
    invNB = sb.tile([128, 4], F32, name="invNB")
    nc.vector.memset(invNB[:], 1.0 / (B * 31))

    # ============== stage 0 + stage A (replicated, full batch) ============
    # Every core computes the full-batch pre-BN activations of the cheap
    # early stages, so all their BatchNorm statistics are local — no
    # AllReduce needed until stage B.  The host permutes the batch per core
    # (own 32 samples first), so "my shard" is always columns 0:32.
    ps_zf = psp.tile([128, B], F32, name="ps_zf", tag="ps")
    nc.tensor.matmul(ps_zf[:], lin_wt, noise_t, start=True, stop=True)
    ps_lab = psp.tile([128, B], F32, name="ps_lab", tag="ps")
    nc.tensor.matmul(ps_lab[:], emb_w, onehot, start=True, stop=True)

    st0 = sb.tile([128, 6], F32, name="st0")
    nc.vector.bn_stats(st0[:], ps_zf[:])
    mv0 = sb.tile([128, 2], F32, name="mv0")
    nc.vector.bn_aggr(mv0[:], st0[:])
    std0 = sb.tile([128, 1], F32, name="std0")
    nc.scalar.activation(std0[:], mv0[:, 1:2], Act.Sqrt, bias=eps_c[:])
    rstd0 = sb.tile([128, 1], F32, name="rstd0")
    nc.vector.reciprocal(rstd0[:], std0[:])
    A0 = sb.tile([128, 1], F32, name="A0")
    nc.vector.tensor_tensor(A0[:], rstd0[:], bn0_g, Alu.mult)
    t0_ = sb.tile([128, 1], F32, name="t0_")
    nc.vector.tensor_tensor(t0_[:], mv0[:, 0:1], A0[:], Alu.mult)
    B0 = sb.tile([128, 1], F32, name="B0")
    nc.vector.tensor_tensor(B0[:], bn0_b, t0_[:], Alu.subtract)

    # latent = [lrelu(bn0(z)) ; emb[label]]  (2 chunks of 128 ch, full batch)
    lat0 = sb.tile([128, B], ADT, name="lat0")
    nc.scalar.activation(lat0[:], ps_zf[:], Act.Identity, bias=B0[:], scale=A0[:])
    nc.vector.scalar_tensor_tensor(lat0[:], lat0[:], 0.01, lat0[:], Alu.mult, Alu.max)
    lat1 = sb.tile([128, B], ADT, name="lat1")
    nc.vector.tensor_copy(lat1[:], ps_lab[:])
    lat = [lat0, lat1]

    # ---- stage A convs (L_in=1): x[co, t, b] = sum_ci w[ci,co,t]*lat[ci,b]
    # full batch per k-block in PSUM; bn_stats per block (equal sizes) then
    # bn_aggr -> local full-batch (mean, var); own shard evacuated to SBUF.
    mvA = sb.tile([128, 12], F32, name="mvA")

    def stageA_conv(wtile, KA, G, kc, mvcol, name):
        outs = []
        for g in range(G):
            nblk = KA // kc
            stS = sb.tile([128, 48], F32, name=f"stA_{name}{g}", tag="stS", bufs=2)
            xdt = F32 if (ADT == BF16 or G > 1) else ADT
            x = sb.tile([128, KA * BC], xdt, name=f"xA_{name}{g}")
            for blk in range(nblk):
                ps = psp.tile([128, kc * B], F32, name=f"psA_{name}{g}_{blk}",
                              tag="ps")
                i = 0
                for kk in range(kc):
                    k = blk * kc + kk
                    # label-embedding chunk first: it has no BN0 dependency,
                    # so the PE can start while the bn0 chain still runs
                    for chunk in (1, 0):
                        idx = ((k * 2 + chunk) * G + g) * 128
                        nc.tensor.matmul(ps[:, kk * B:(kk + 1) * B],
                                         wtile[:, idx:idx + 128], lat[chunk][:],
                                         start=(i == 0), stop=(i == 2 * kc - 1))
                        i += 1
                nc.vector.bn_stats(stS[:, blk * 6:(blk + 1) * 6], ps[:])
                src = ps[:].rearrange("p (kk b) -> p kk b", b=B)[:, :, 0:BC]
                dst = x[:].rearrange("p (t b) -> p t b", b=BC)[
                    :, blk * kc:(blk + 1) * kc, :]
                nc.scalar.copy(dst, src)
            nc.vector.bn_aggr(mvA[:, (mvcol + g) * 2:(mvcol + g) * 2 + 2], stS[:, 0:nblk * 6])
            outs.append(x)
        return outs

    xA_h = stageA_conv(wA_h, 16, 1, 2, 0, "h")[0]
    xA_w = stageA_conv(wA_w, 16, 1, 2, 1, "w")[0]

    # ---- per-channel BN coefficients: A = g/sqrt(var+eps), B = be - mean*A
    def bn_from_mv(mean_v, var_v, ncols, g_t, be_t, name):
        std = sb.tile([128, ncols], F32, name=f"std{name}")
        nc.scalar.activation(std[:], var_v, Act.Sqrt, bias=eps_c[:])
        rstd = sb.tile([128, ncols], F32, name=f"rstd{name}")
        nc.vector.reciprocal(rstd[:], std[:])
        Atl = sb.tile([128, ncols], F32, name=f"A{name}")
        nc.vector.tensor_tensor(Atl[:], rstd[:], g_t, Alu.mult)
        tmp = sb.tile([128, ncols], F32, name=f"tmp{name}")
        nc.vector.tensor_tensor(tmp[:], mean_v, Atl[:], Alu.mult)
        Btl = sb.tile([128, ncols], F32, name=f"B{name}")
        nc.vector.tensor_tensor(Btl[:], be_t, tmp[:], Alu.subtract)
        return Atl, Btl

    def bn_coeffs(stg, ncols, invN, g_t, be_t, name):
        sc = sb.tile([128, 2 * ncols], F32, name=f"sc{name}")
        nc.vector.tensor_tensor(sc[:], stg[:], invN[:], Alu.mult)
        mean = sc[:, 0:ncols]
        ex2 = sc[:, ncols:2 * ncols]
        msq = sb.tile([128, ncols], F32, name=f"msq{name}")
        nc.scalar.activation(msq[:], mean, Act.Square)
        var = sb.tile([128, ncols], F32, name=f"var{name}")
        nc.vector.tensor_tensor(var[:], ex2, msq[:], Alu.subtract)
        return bn_from_mv(mean, var[:], ncols, g_t, be_t, name)

    # independent per-path coefficient chains so each conv path unblocks as
    # soon as its own stats are in
    A_Ah, B_Ah = bn_from_mv(mvA[:, 0:1], mvA[:, 1:2], 1, gA[:, 0:1], beA[:, 0:1], "Ah")
    A_Aw, B_Aw = bn_from_mv(mvA[:, 2:3], mvA[:, 3:4], 1, gA[:, 1:2], beA[:, 1:2], "Aw")


    def bn_apply(dst, src, Atl, Btl, col, slope):
        nc.scalar.activation(dst, src, Act.Identity,
                             bias=Btl[:, col:col + 1], scale=Atl[:, col:col + 1])
        nc.vector.scalar_tensor_tensor(dst, dst, slope, dst, Alu.mult, Alu.max)

    if ADT != BF16:
        h1, w1 = xA_h, xA_w
        bn_apply(h1[:], h1[:], A_Ah, B_Ah, 0, 0.2)
        bn_apply(w1[:], w1[:], A_Aw, B_Aw, 0, 0.2)
    else:
        h1 = sb.tile([128, 16 * BC], ADT, name="h1")
        bn_apply(h1[:], xA_h[:], A_Ah, B_Ah, 0, 0.2)
        w1 = sb.tile([128, 16 * BC], ADT, name="w1")
        bn_apply(w1[:], xA_w[:], A_Aw, B_Aw, 0, 0.2)


    # ======================= stage B convs (16 -> 31) =====================
    # y[co, t, b] += sum_ci w2[ci, co, k] * h1[ci, t-k, b]

    def convT(wtile, src_chunks, LI, LO, KK, G, psname, n_bh=2, widx=None):
        """shifted-window convT: returns psum tiles [(g, bh) -> [128, LO*16]]

        Default emits one full-window matmul per (k, chunk) — on HW the
        per-element has_written bits make partially-overlapping windows
        accumulate correctly.  CoreSim asserts uniform pending state per
        matmul, so SIM_SAFE mode splits each k>=1 window into an
        all-covered part plus a single fresh output column.
        """
        pss = {}
        n_ck = len(src_chunks)
        for g in range(G):
            for bh in range(n_bh):
                ps = psp.tile([128, LO * 16], F32, name=f"ps{psname}_{g}_{bh}",
                              tag="ps")
                i = 0
                for k in range(KK):
                    for ck in range(n_ck):
                        lw = wtile[:, widx(ck, k, g):widx(ck, k, g) + 128]
                        src = src_chunks[ck]
                        last = (k == KK - 1 and ck == n_ck - 1)
                        if k == 0 or not SIM_SAFE:
                            rhs = src[:].rearrange("p (t b) -> p t b", b=BC)[
                                :, 0:LI, bh * 16:(bh + 1) * 16]
                            nc.tensor.matmul(ps[:, k * 16:(k + LI) * 16], lw, rhs,
                                             start=(i == 0), stop=last)
                            i += 1
                        else:
                            rhs = src[:].rearrange("p (t b) -> p t b", b=BC)[
                                :, 0:LI - 1, bh * 16:(bh + 1) * 16]
                            nc.tensor.matmul(
                                ps[:, k * 16:(k + LI - 1) * 16], lw, rhs,
                                start=False, stop=False)
                            i += 1
                            rhs2 = src[:, (LI - 1) * BC + bh * 16:
                                       (LI - 1) * BC + bh * 16 + 16]
                            nc.tensor.matmul(
                                ps[:, (k + LI - 1) * 16:(k + LI) * 16], lw, rhs2,
                                start=False, stop=last)
                            i += 1
                pss[(g, bh)] = ps
        return pss

    psB_h = convT(wB_h, [h1], 16, 31, 16, 2, "Bh",
                  widx=lambda ck, k, g: (k * 2 + g) * 128)
    psB_w = convT(wB_w, [w1], 16, 31, 16, 2, "Bw",
                  widx=lambda ck, k, g: (k * 2 + g) * 128)

    # evacuate + stats; xB tiles are [128, 31*BC] t-major over full b.
    # Each path (h, w) gets its own small AllReduce so the h-path collective
    # overlaps the w-path convs (and vice versa for stage C).
    def evacB(pss, name):
        stT = sb.tile([128, 8], F32, name=f"stB{name}")
        outs = []
        for g in range(2):
            xdt = F32 if ADT == BF16 else ADT
            x = sb.tile([128, 31 * BC], xdt, name=f"xB_{name}{g}")
            for bh in range(2):
                ps = pss[(g, bh)]
                col = g * 2 + bh
                dstv = x[:].rearrange("p (t b) -> p t b", b=BC)[:, :, bh * 16:(bh + 1) * 16]
                nc.vector.tensor_scalar(dstv, ps[:], 1.0, None, Alu.mult, Alu.add,
                                        accum_out=stT[:, col:col + 1])
                scr = sb.tile([128, 512], F32, name=f"scrB_{name}{g}{bh}",
                              tag="scr", bufs=1)
                nc.scalar.activation(scr[:, 0:31 * 16], ps[:], Act.Square,
                                     accum_out=stT[:, 4 + col:5 + col])
            outs.append(x)
        return outs, stT

    def allreduceB(stT, name):
        ar_in = dram.tile([128, 8], F32, name=f"arB{name}_in")
        ar_out = dram.tile([128, 8], F32, name=f"arB{name}_out")
        nc.sync.dma_start(ar_in[:], stT[:])
        if NO_AR:
            nc.gpsimd.dma_start(ar_out[:], ar_in[:])
        else:
            nc.gpsimd.collective_compute("AllReduce", Alu.add,
                                         replica_groups=[list(range(N_CORES))],
                                         ins=[ar_in.opt()], outs=[ar_out.opt()])
        stg = sb.tile([128, 8], F32, name=f"stBg{name}")
        nc.sync.dma_start(stg[:], ar_out[:])
        # combine the two batch-half partial sums: cols (s1 g0, s1 g1, s2 g0, s2 g1)
        sts = sb.tile([128, 4], F32, name=f"stBs{name}")
        v = stg[:].rearrange("p (c two) -> p c two", two=2)
        nc.vector.tensor_tensor(sts[:], v[:, :, 0:1].squeeze(2),
                                v[:, :, 1:2].squeeze(2), Alu.add)
        return sts

    def applyB(xs, A_t, B_t, name):
        outs = []
        for g in range(2):
            if ADT != BF16:
                t = xs[g]
                bn_apply(t[:], t[:], A_t, B_t, g, 0.2)
            else:
                t = sb.tile([128, 31 * BC], ADT, name=f"{name}2_{g}")
                bn_apply(t[:], xs[g][:], A_t, B_t, g, 0.2)
            outs.append(t)
        return outs

    xB_h, stBh = evacB(psB_h, "h")
    stsH = allreduceB(stBh, "h")
    xB_w, stBw = evacB(psB_w, "w")
    stsW = allreduceB(stBw, "w")

    # c-path stage-A convs + coefficients, scheduled into the AllReduce gaps
    xA_c = stageA_conv(wA_c, 3, 4, 1, 2, "c")
    mvc = mvA[:].rearrange("p (c two) -> p c two", two=2)[:, 2:6, :]
    A_Ac, B_Ac = bn_from_mv(mvc[:, :, 0:1].squeeze(2), mvc[:, :, 1:2].squeeze(2),
                            4, gA[:, 2:6], beA[:, 2:6], "Ac")
    nc.vector.tensor_tensor(A_Ac[:], A_Ac[:], coef_p, Alu.mult)
    nc.vector.tensor_tensor(B_Ac[:], B_Ac[:], coef_p, Alu.mult)
    cT = []
    for g in range(4):
        ct = sb.tile([128, 3 * BC], F32, name=f"cT{g}")
        src = xA_c[g][:].rearrange("p (c b) -> p c b", b=BC)
        dstv = ct[:].rearrange("p (b c) -> p c b", c=3)
        bn_apply(dstv, src, A_Ac, B_Ac, g, 0.2)
        cT.append(ct)
    A_Bh, B_Bh = bn_coeffs(stsH, 2, invNB, gB[:, 0:2], beB[:, 0:2], "Bsth")
    h2 = applyB(xB_h, A_Bh, B_Bh, "h")
    A_Bw, B_Bw = bn_coeffs(stsW, 2, invNB, gB[:, 2:4], beB[:, 2:4], "Bstw")
    w2 = applyB(xB_w, A_Bw, B_Bw, "w")

    # ======================= stage C convs (31 -> 32) =====================
    def stageC(wtile, src_chunks, b3t, name, out_dt):
        pss = convT(wtile, src_chunks, 31, 32, 2, 4, name,
                    widx=lambda ck, k, g: ((k * 2 + ck) * 4 + g) * 128)
        outs = []
        for g in range(4):
            h = sb.tile([128, BC * 32], out_dt, name=f"{name}3_{g}")
            for bh in range(2):
                ps = pss[(g, bh)]
                # psum cols t*16+j -> sbuf cols (bh*16+j)*32 + t
                inv = ps[:].rearrange("p (t j) -> p j t", j=16)
                dstv = h[:].rearrange("p (b t) -> p b t", t=32)[
                    :, bh * 16:(bh + 1) * 16, :]
                nc.scalar.activation(dstv, inv, Act.Tanh, bias=b3t[:, g:g + 1])
            outs.append(h)
        return outs

    h3 = stageC(wC_h, h2, b3h, "h", F32)
    GDTt = GDT
    w3 = stageC(wC_w, w2, b3w, "w", GDTt)

    # ======================= rank-weighted outer-product head =============
    # G[q][r, b*96 + ci*32 + hi] = c~[q][r, b*3+ci] * h3[q][r, b*32+hi]
    Gt = []
    for q in range(4):
        g = sb.tile([128, BC * 96], GDTt, name=f"G{q}")
        cv = cT[q][:].rearrange("p (b c) -> p b c", c=3).unsqueeze(3) \
            .broadcast_to([128, BC, 3, 32])
        hv = h3[q][:].rearrange("p (b h) -> p b h", h=32).unsqueeze(2) \
            .broadcast_to([128, BC, 3, 32])
        gv = g[:].rearrange("p (b c h) -> p b c h", c=3, h=32)
        eng = nc.gpsimd if q == 3 else nc.vector
        eng.tensor_tensor(gv, cv, hv, Alu.mult)
        Gt.append(g)

    outsb = sb.tile([96, BC * 32], F32, name="outsb")
    out_v = out_ap.rearrange("b c h w -> (c h) b w")
    for grp in range(BC // 4):
        po = psp.tile([96, 4 * 32], F32, name=f"po{grp}", tag="ps")
        for q in range(4):
            for j in range(4):
                bb = grp * 4 + j
                nc.tensor.matmul(po[:, j * 32:(j + 1) * 32],
                                 Gt[q][:, bb * 96:(bb + 1) * 96],
                                 w3[q][:, bb * 32:(bb + 1) * 32],
                                 start=(j == 0 and q == 0),
                                 stop=(j == 3 and q == 3))
        nc.vector.tensor_copy(outsb[:, grp * 128:(grp + 1) * 128], po[:])
        if grp % 4 == 3:
            # stream each 16-sample half out as soon as it is evacuated
            h0 = (grp - 3) * 4
            nc.sync.dma_start(
                out_v[:, h0:h0 + 16, :],
                outsb[:, h0 * 32:(h0 + 16) * 32].rearrange("p (b w) -> p b w", w=32))

    sb.release()
    psp.release()
    dram.release()


# --------------------------------------------------------------------------
# host side
# --------------------------------------------------------------------------

def _build_module():
    nc = bacc.Bacc("TRN2", target_bir_lowering=False, debug=False,
                   num_devices=N_CORES)
    WDT = _cdt()
    specs = {
        "p100": ((NOISE, 128 + B), F32),
        "p10": ((NCLASS, B + 128), F32),
        "p128": ((128, 34), F32),
        "wA_c": ((128, 3072), WDT), "wA_h": ((128, 4096), WDT),
        "wA_w": ((128, 4096), WDT),
        "wB_h": ((128, 4096), WDT), "wB_w": ((128, 4096), WDT),
        "wC_h": ((128, 2048), WDT), "wC_w": ((128, 2048), WDT),
    }
    I = {}
    for name, (shape, dt) in specs.items():
        I[name] = nc.dram_tensor(name, list(shape), dt, kind="ExternalInput").ap()
    out = nc.dram_tensor("out", [BC, 3, 32, 32], F32, kind="ExternalOutput")
    with tile.TileContext(nc) as tc:
        _emit(nc, tc, I, out.ap())
    nc.compile()
    return nc


def _np(x):
    return np.ascontiguousarray(np.asarray(x, dtype=np.float32))


def _pack_inputs(inputs):
    """host-side layout packing -> (replicated dict, per-core dicts)"""
    wnp = np.dtype(mybir.dt.np(_cdt()))
    noise = _np(inputs["noise"])
    label = np.asarray(inputs["label"]).astype(np.int64)

    c_w1 = _np(inputs["c_w1"])   # (256, 512, 3)
    h_w1 = _np(inputs["h_w1"])   # (256, 128, 16)
    w_w1 = _np(inputs["w_w1"])
    h_w2 = _np(inputs["h_w2"])   # (128, 256, 16)
    w_w2 = _np(inputs["w_w2"])
    h_w3 = _np(inputs["h_w3"])   # (256, 512, 2)
    w_w3 = _np(inputs["w_w3"])

    def packA_c(w):   # -> [ci_in, (k, chunk, g, co_in)]
        return np.ascontiguousarray(
            w.reshape(2, 128, 4, 128, 3).transpose(1, 4, 0, 2, 3).reshape(128, -1))

    def packA_h(w):   # (256,128,16) -> [ci_in, (k, chunk, co)]
        return np.ascontiguousarray(
            w.reshape(2, 128, 128, 16).transpose(1, 3, 0, 2).reshape(128, -1))

    def packB(w):     # (128,256,16) -> [ci, (k, g, co_in)]
        return np.ascontiguousarray(
            w.reshape(128, 2, 128, 16).transpose(0, 3, 1, 2).reshape(128, -1))

    def packC(w):     # (256,512,2) -> [ci_in, (k, chunk, g, co_in)]
        return np.ascontiguousarray(
            w.reshape(2, 128, 4, 128, 2).transpose(1, 4, 0, 2, 3).reshape(128, -1))

    def col128(*arrs):
        return np.ascontiguousarray(
            np.concatenate([a.reshape(-1, 128).T for a in arrs], axis=1))

    p128 = np.concatenate([
        _np(inputs["bn0_g"]).reshape(128, 1),
        _np(inputs["bn0_b"]).reshape(128, 1),
        col128(_np(inputs["h_g1"]), _np(inputs["w_g1"]), _np(inputs["c_g1"])),
        col128(_np(inputs["h_be1"]), _np(inputs["w_be1"]), _np(inputs["c_be1"])),
        col128(_np(inputs["coef"])),
        col128(_np(inputs["h_g2"]), _np(inputs["w_g2"])),
        col128(_np(inputs["h_be2"]), _np(inputs["w_be2"])),
        col128(_np(inputs["h_b3"])),
        col128(_np(inputs["w_b3"])),
    ], axis=1)

    rep = {
        "p128": np.ascontiguousarray(p128),
        "wA_c": packA_c(c_w1).astype(wnp),
        "wA_h": packA_h(h_w1).astype(wnp),
        "wA_w": packA_h(w_w1).astype(wnp),
        "wB_h": packB(h_w2).astype(wnp),
        "wB_w": packB(w_w2).astype(wnp),
        "wC_h": packC(h_w3).astype(wnp),
        "wC_w": packC(w_w3).astype(wnp),
    }

    lin_wt = _np(inputs["lin_w"]).T
    emb_w = _np(inputs["emb"])
    noise_t = noise.T
    per_core = []
    for c in range(N_CORES):
        own = np.arange(c * BC, (c + 1) * BC)
        rest = np.delete(np.arange(B), own)
        perm = np.concatenate([own, rest])
        oh = (label[perm][None, :] == np.arange(NCLASS)[:, None]).astype(np.float32)
        per_core.append({
            "p100": np.ascontiguousarray(
                np.concatenate([lin_wt, noise_t[:, perm]], axis=1)),
            "p10": np.ascontiguousarray(np.concatenate([oh, emb_w], axis=1)),
            **rep,
        })
    return per_core


def kernel(**inputs) -> np.ndarray:
    with _lock:
        key = (PREC, NO_AR, SIM_SAFE)
        nc = _cache.get(key)
        if nc is None:
            nc = _build_module()
            _cache[key] = nc
    in_maps = _pack_inputs(inputs)
    res = run_bass_kernel_spmd(nc, in_maps, core_ids=list(range(N_CORES)))
    return np.concatenate([r["out"] for r in res.results], axis=0)


# revision 34
# speedup vs baseline: 21122.4035x; 1.0742x over previous
"""Trainium2 Bass kernel for nn_DecompGen (conditional generator + rank-decomposed
outer-product head).

Sharding: data parallelism — batch B=256 is split 32-per-core across 8
NeuronCores; the small parameter set is replicated.

Training-mode BatchNorm needs full-batch statistics.  The cheap early stages
(input linear + the three L_in=1 convs) are therefore computed REPLICATED at
full batch on every core — their BN stats are then local (bn_stats/bn_aggr),
and the host permutes the batch per core (own 32 samples first) so each
core's shard is always columns 0:32 with no runtime indexing.  Only the two
expensive stage-B convs run sharded; their per-channel sum/sum-of-squares
stats take one tiny AllReduce per path (h, w), each overlapping the other
path's compute.

Layout conventions (per core, channels on SBUF partitions):
  - conv activations are "t-major": tile column = t*32 + b
  - convT accumulates K shifted matmuls into one PSUM bank; weight-tap k
    covers output window t in [k, k+L_in).  On HW the per-element PSUM
    has_written bits make overlapping windows accumulate correctly; CoreSim
    requires uniform pending state per matmul, so KERNEL_SIM_SAFE=1 splits
    each k>=1 window into an all-covered part plus the single fresh column.
  - float32r (fp32 storage, PE fast path) is used for the conv chain; the
    output head and all BN math stay fp32.  Measured absmax-relative error
    vs the fp32 reference: ~5.6e-4.
  - the rank-weighted outer-product head runs per-sample matmuls
    out[(ci,hi), wi] = sum_r (coef*c)[r,ci]*h[r,hi] . w[r,wi]
"""

import os
import threading

import numpy as np

import concourse.bacc as bacc
import concourse.mybir as mybir
import concourse.tile as tile
from concourse.bass_utils import run_bass_kernel_spmd

F32 = mybir.dt.float32
F32R = mybir.dt.float32r
BF16 = mybir.dt.bfloat16

N_CORES = 8
B, NOISE, NCLASS, RANK = 256, 100, 10, 512
R4, R2 = RANK // 4, RANK // 2  # 128, 256
BC = B // N_CORES  # 32 samples per core
EPS = 1e-5

# precision config: "f32" (exact), "f32r" (fp32 storage, fast PE mode on the
# big convs), "wbf16" (bf16 weights + f32r activations), "bf16" (bf16
# weights/activations on the conv + einsum path)
PREC = os.environ.get("KERNEL_PREC", "f32r")
NO_AR = os.environ.get("KERNEL_NO_AR", "0") == "1"
SIM_SAFE = os.environ.get("KERNEL_SIM_SAFE", "0") == "1"

_lock = threading.Lock()
_cache: dict = {}


def _cdt():
    """storage dtype of conv WEIGHTS"""
    if PREC in ("bf16", "wbf16"):
        return BF16
    if PREC == "f32r":
        return F32R
    return F32


def _adt():
    """storage dtype of conv ACTIVATIONS (matmul moving operands)"""
    if PREC == "bf16":
        return BF16
    if PREC in ("f32r", "wbf16"):
        return F32R
    return F32


# --------------------------------------------------------------------------
# device kernel
# --------------------------------------------------------------------------

def _emit(nc, tc, I, out_ap):
    ADT = _adt()          # conv activations (h1/h2, latent)
    WDT = _cdt()          # conv weights (host already cast)
    # einsum G / w3 dtype: the head matmuls are weight-load bound either
    # way (M=96 < 128 disables FWL), so fp32 costs ~nothing and removes the
    # dominant bf16 rounding term from the output.
    GDT = F32
    Alu = mybir.AluOpType
    Act = mybir.ActivationFunctionType

    sb = tc.alloc_tile_pool(name="sb", bufs=1)
    # all PSUM tiles are <= one bank; share 8 rotating bank slots
    psp = tc.alloc_tile_pool(name="ps", bufs=8, space="PSUM")
    dram = tc.alloc_tile_pool(name="dram", bufs=1, space="DRAM")

    def load(name, shape, dtype=F32, eng=None, chunks=1):
        t = sb.tile(list(shape), dtype, name=f"sb_{name}")
        eng = eng or nc.sync
        n = shape[1]
        step = (n + chunks - 1) // chunks
        for c0 in range(0, n, step):
            c1 = min(n, c0 + step)
            eng.dma_start(t[:, c0:c1], I[name][:, c0:c1])
        return t

    # ---- input loads: small tensors ride in 3 packed arrays (one DMA each
    # instead of ~13 — every HWDGE dma pays ~1us of queue latency) ----
    p100 = load("p100", (NOISE, 128 + B))
    lin_wt = p100[:, 0:128]
    noise_t = p100[:, 128:128 + B]
    p10 = load("p10", (NCLASS, B + 128))
    onehot = p10[:, 0:B]
    emb_w = p10[:, B:B + 128]
    p128 = load("p128", (128, 34))
    bn0_g = p128[:, 0:1]
    bn0_b = p128[:, 1:2]
    gA = p128[:, 2:8]
    beA = p128[:, 8:14]
    coef_p = p128[:, 14:18]
    gB = p128[:, 18:22]
    beB = p128[:, 22:26]
    b3h = p128[:, 26:30]
    b3w = p128[:, 30:34]
    # stage A+B weights: h/w chunks interleaved on one queue so both conv
    # paths stream in at the same rate (PE consumes them alternately)
    wA_h = sb.tile([128, 2 * 16 * 128], WDT, name="sb_wA_h")
    wA_w = sb.tile([128, 2 * 16 * 128], WDT, name="sb_wA_w")
    wB_h = sb.tile([128, 16 * 2 * 128], WDT, name="sb_wB_h")
    wB_w = sb.tile([128, 16 * 2 * 128], WDT, name="sb_wB_w")
    # h-path weights stream first: conv2-h is the first big PE consumer
    for c0 in range(0, 4096, 1024):
        nc.sync.dma_start(wA_h[:, c0:c0 + 1024], I["wA_h"][:, c0:c0 + 1024])
    for c0 in range(0, 4096, 1024):
        nc.sync.dma_start(wB_h[:, c0:c0 + 1024], I["wB_h"][:, c0:c0 + 1024])
    for c0 in range(0, 4096, 1024):
        nc.sync.dma_start(wA_w[:, c0:c0 + 1024], I["wA_w"][:, c0:c0 + 1024])
    for c0 in range(0, 4096, 1024):
        nc.sync.dma_start(wB_w[:, c0:c0 + 1024], I["wB_w"][:, c0:c0 + 1024])
    # c path feeds the AllReduce-window filler work (~60us in); conv3 weights
    # are needed a bit later
    wA_c = load("wA_c", (128, 2 * 3 * 4 * 128), WDT, eng=nc.gpsimd, chunks=2)
    wC_h = load("wC_h", (128, 2 * 2 * 4 * 128), WDT, eng=nc.gpsimd, chunks=2)
    wC_w = load("wC_w", (128, 2 * 2 * 4 * 128), WDT, eng=nc.scalar, chunks=2)

    eps_c = sb.tile([128, 1], F32, name="eps_c")
    nc.vector.memset(eps_c[:], EPS)
    invNB = sb.tile([128, 4], F32, name="invNB")
    nc.vector.memset(invNB[:], 1.0 / (B * 31))

    # ============== stage 0 + stage A (replicated, full batch) ============
    # Every core computes the full-batch pre-BN activations of the cheap
    # early stages, so all their BatchNorm statistics are local — no
    # AllReduce needed until stage B.  The host permutes the batch per core
    # (own 32 samples first), so "my shard" is always columns 0:32.
    ps_zf = psp.tile([128, B], F32, name="ps_zf", tag="ps")
    nc.tensor.matmul(ps_zf[:], lin_wt, noise_t, start=True, stop=True)
    ps_lab = psp.tile([128, B], F32, name="ps_lab", tag="ps")
    nc.tensor.matmul(ps_lab[:], emb_w, onehot, start=True, stop=True)

    st0 = sb.tile([128, 6], F32, name="st0")
    nc.vector.bn_stats(st0[:], ps_zf[:])
    mv0 = sb.tile([128, 2], F32, name="mv0")
    nc.vector.bn_aggr(mv0[:], st0[:])
    std0 = sb.tile([128, 1], F32, name="std0")
    nc.scalar.activation(std0[:], mv0[:, 1:2], Act.Sqrt, bias=eps_c[:])
    rstd0 = sb.tile([128, 1], F32, name="rstd0")
    nc.vector.reciprocal(rstd0[:], std0[:])
    A0 = sb.tile([128, 1], F32, name="A0")
    nc.vector.tensor_tensor(A0[:], rstd0[:], bn0_g, Alu.mult)
    t0_ = sb.tile([128, 1], F32, name="t0_")
    nc.vector.tensor_tensor(t0_[:], mv0[:, 0:1], A0[:], Alu.mult)
    B0 = sb.tile([128, 1], F32, name="B0")
    nc.vector.tensor_tensor(B0[:], bn0_b, t0_[:], Alu.subtract)

    # latent = [lrelu(bn0(z)) ; emb[label]]  (2 chunks of 128 ch, full batch)
    lat0 = sb.tile([128, B], ADT, name="lat0")
    nc.scalar.activation(lat0[:], ps_zf[:], Act.Identity, bias=B0[:], scale=A0[:])
    nc.vector.scalar_tensor_tensor(lat0[:], lat0[:], 0.01, lat0[:], Alu.mult, Alu.max)
    lat1 = sb.tile([128, B], ADT, name="lat1")
    nc.vector.tensor_copy(lat1[:], ps_lab[:])
    lat = [lat0, lat1]

    # ---- stage A convs (L_in=1): x[co, t, b] = sum_ci w[ci,co,t]*lat[ci,b]
    # full batch per k-block in PSUM; bn_stats per block (equal sizes) then
    # bn_aggr -> local full-batch (mean, var); own shard evacuated to SBUF.
    mvA = sb.tile([128, 12], F32, name="mvA")

    def stageA_conv(wtile, KA, G, kc, mvcol, name, g_range=None):
        outs = []
        for g in (g_range if g_range is not None else range(G)):
            nblk = KA // kc
            stS = sb.tile([128, 48], F32, name=f"stA_{name}{g}", tag="stS", bufs=2)
            xdt = F32 if (ADT == BF16 or G > 1) else ADT
            x = sb.tile([128, KA * BC], xdt, name=f"xA_{name}{g}")
            for blk in range(nblk):
                ps = psp.tile([128, kc * B], F32, name=f"psA_{name}{g}_{blk}",
                              tag="ps")
                i = 0
                for kk in range(kc):
                    k = blk * kc + kk
                    # label-embedding chunk first: it has no BN0 dependency,
                    # so the PE can start while the bn0 chain still runs
                    for chunk in (1, 0):
                        idx = ((k * 2 + chunk) * G + g) * 128
                        nc.tensor.matmul(ps[:, kk * B:(kk + 1) * B],
                                         wtile[:, idx:idx + 128], lat[chunk][:],
                                         start=(i == 0), stop=(i == 2 * kc - 1))
                        i += 1
                nc.vector.bn_stats(stS[:, blk * 6:(blk + 1) * 6], ps[:])
                src = ps[:].rearrange("p (kk b) -> p kk b", b=B)[:, :, 0:BC]
                dst = x[:].rearrange("p (t b) -> p t b", b=BC)[
                    :, blk * kc:(blk + 1) * kc, :]
                nc.scalar.copy(dst, src)
            nc.vector.bn_aggr(mvA[:, (mvcol + g) * 2:(mvcol + g) * 2 + 2],
                              stS[:, 0:nblk * 6])
            outs.append(x)
        return outs

    xA_h = stageA_conv(wA_h, 16, 1, 2, 0, "h")[0]
    xA_w = stageA_conv(wA_w, 16, 1, 2, 1, "w")[0]

    # ---- per-channel BN coefficients: A = g/sqrt(var+eps), B = be - mean*A
    def bn_from_mv(mean_v, var_v, ncols, g_t, be_t, name):
        std = sb.tile([128, ncols], F32, name=f"std{name}")
        nc.scalar.activation(std[:], var_v, Act.Sqrt, bias=eps_c[:])
        rstd = sb.tile([128, ncols], F32, name=f"rstd{name}")
        nc.vector.reciprocal(rstd[:], std[:])
        Atl = sb.tile([128, ncols], F32, name=f"A{name}")
        nc.vector.tensor_tensor(Atl[:], rstd[:], g_t, Alu.mult)
        tmp = sb.tile([128, ncols], F32, name=f"tmp{name}")
        nc.vector.tensor_tensor(tmp[:], mean_v, Atl[:], Alu.mult)
        Btl = sb.tile([128, ncols], F32, name=f"B{name}")
        nc.vector.tensor_tensor(Btl[:], be_t, tmp[:], Alu.subtract)
        return Atl, Btl

    def bn_coeffs(stg, ncols, invN, g_t, be_t, name):
        sc = sb.tile([128, 2 * ncols], F32, name=f"sc{name}")
        nc.vector.tensor_tensor(sc[:], stg[:], invN[:], Alu.mult)
        mean = sc[:, 0:ncols]
        ex2 = sc[:, ncols:2 * ncols]
        msq = sb.tile([128, ncols], F32, name=f"msq{name}")
        nc.scalar.activation(msq[:], mean, Act.Square)
        var = sb.tile([128, ncols], F32, name=f"var{name}")
        nc.vector.tensor_tensor(var[:], ex2, msq[:], Alu.subtract)
        return bn_from_mv(mean, var[:], ncols, g_t, be_t, name)

    # independent per-path coefficient chains so each conv path unblocks as
    # soon as its own stats are in
    A_Ah, B_Ah = bn_from_mv(mvA[:, 0:1], mvA[:, 1:2], 1, gA[:, 0:1], beA[:, 0:1], "Ah")
    A_Aw, B_Aw = bn_from_mv(mvA[:, 2:3], mvA[:, 3:4], 1, gA[:, 1:2], beA[:, 1:2], "Aw")


    def bn_apply(dst, src, Atl, Btl, col, slope):
        nc.scalar.activation(dst, src, Act.Identity,
                             bias=Btl[:, col:col + 1], scale=Atl[:, col:col + 1])
        nc.vector.scalar_tensor_tensor(dst, dst, slope, dst, Alu.mult, Alu.max)

    if ADT != BF16:
        h1, w1 = xA_h, xA_w
        bn_apply(h1[:], h1[:], A_Ah, B_Ah, 0, 0.2)
        bn_apply(w1[:], w1[:], A_Aw, B_Aw, 0, 0.2)
    else:
        h1 = sb.tile([128, 16 * BC], ADT, name="h1")
        bn_apply(h1[:], xA_h[:], A_Ah, B_Ah, 0, 0.2)
        w1 = sb.tile([128, 16 * BC], ADT, name="w1")
        bn_apply(w1[:], xA_w[:], A_Aw, B_Aw, 0, 0.2)


    # ======================= stage B convs (16 -> 31) =====================
    # y[co, t, b] += sum_ci w2[ci, co, k] * h1[ci, t-k, b]

    def convT(wtile, src_chunks, LI, LO, KK, G, psname, n_bh=2, widx=None):
        """shifted-window convT: returns psum tiles [(g, bh) -> [128, LO*16]]

        Default emits one full-window matmul per (k, chunk) — on HW the
        per-element has_written bits make partially-overlapping windows
        accumulate correctly.  CoreSim asserts uniform pending state per
        matmul, so SIM_SAFE mode splits each k>=1 window into an
        all-covered part plus a single fresh output column.
        """
        pss = {}
        n_ck = len(src_chunks)
        for g in range(G):
            for bh in range(n_bh):
                ps = psp.tile([128, LO * 16], F32, name=f"ps{psname}_{g}_{bh}",
                              tag="ps")
                i = 0
                for k in range(KK):
                    for ck in range(n_ck):
                        lw = wtile[:, widx(ck, k, g):widx(ck, k, g) + 128]
                        src = src_chunks[ck]
                        last = (k == KK - 1 and ck == n_ck - 1)
                        if k == 0 or not SIM_SAFE:
                            rhs = src[:].rearrange("p (t b) -> p t b", b=BC)[
                                :, 0:LI, bh * 16:(bh + 1) * 16]
                            nc.tensor.matmul(ps[:, k * 16:(k + LI) * 16], lw, rhs,
                                             start=(i == 0), stop=last)
                            i += 1
                        else:
                            rhs = src[:].rearrange("p (t b) -> p t b", b=BC)[
                                :, 0:LI - 1, bh * 16:(bh + 1) * 16]
                            nc.tensor.matmul(
                                ps[:, k * 16:(k + LI - 1) * 16], lw, rhs,
                                start=False, stop=False)
                            i += 1
                            rhs2 = src[:, (LI - 1) * BC + bh * 16:
                                       (LI - 1) * BC + bh * 16 + 16]
                            nc.tensor.matmul(
                                ps[:, (k + LI - 1) * 16:(k + LI) * 16], lw, rhs2,
                                start=False, stop=last)
                            i += 1
                pss[(g, bh)] = ps
        return pss

    psB_h = convT(wB_h, [h1], 16, 31, 16, 2, "Bh",
                  widx=lambda ck, k, g: (k * 2 + g) * 128)
    psB_w = convT(wB_w, [w1], 16, 31, 16, 2, "Bw",
                  widx=lambda ck, k, g: (k * 2 + g) * 128)

    # evacuate + stats; xB tiles are [128, 31*BC] t-major over full b.
    # Each path (h, w) gets its own small AllReduce so the h-path collective
    # overlaps the w-path convs (and vice versa for stage C).
    def evacB(pss, name):
        stT = sb.tile([128, 8], F32, name=f"stB{name}")
        outs = []
        for g in range(2):
            xdt = F32 if ADT == BF16 else ADT
            x = sb.tile([128, 31 * BC], xdt, name=f"xB_{name}{g}")
            for bh in range(2):
                ps = pss[(g, bh)]
                col = g * 2 + bh
                dstv = x[:].rearrange("p (t b) -> p t b", b=BC)[:, :, bh * 16:(bh + 1) * 16]
                nc.vector.tensor_scalar(dstv, ps[:], 1.0, None, Alu.mult, Alu.add,
                                        accum_out=stT[:, col:col + 1])
                scr = sb.tile([128, 512], F32, name=f"scrB_{name}{g}{bh}",
                              tag="scr", bufs=1)
                nc.scalar.activation(scr[:, 0:31 * 16], ps[:], Act.Square,
                                     accum_out=stT[:, 4 + col:5 + col])
            outs.append(x)
        return outs, stT

    def allreduceB(stT, name):
        ar_in = dram.tile([128, 8], F32, name=f"arB{name}_in")
        ar_out = dram.tile([128, 8], F32, name=f"arB{name}_out")
        nc.sync.dma_start(ar_in[:], stT[:])
        if NO_AR:
            nc.gpsimd.dma_start(ar_out[:], ar_in[:])
        else:
            nc.gpsimd.collective_compute("AllReduce", Alu.add,
                                         replica_groups=[list(range(N_CORES))],
                                         ins=[ar_in.opt()], outs=[ar_out.opt()])
        stg = sb.tile([128, 8], F32, name=f"stBg{name}")
        nc.sync.dma_start(stg[:], ar_out[:])
        # combine the two batch-half partial sums: cols (s1 g0, s1 g1, s2 g0, s2 g1)
        sts = sb.tile([128, 4], F32, name=f"stBs{name}")
        v = stg[:].rearrange("p (c two) -> p c two", two=2)
        nc.vector.tensor_tensor(sts[:], v[:, :, 0:1].squeeze(2),
                                v[:, :, 1:2].squeeze(2), Alu.add)
        return sts

    def applyB(xs, A_t, B_t, name):
        outs = []
        for g in range(2):
            if ADT != BF16:
                t = xs[g]
                bn_apply(t[:], t[:], A_t, B_t, g, 0.2)
            else:
                t = sb.tile([128, 31 * BC], ADT, name=f"{name}2_{g}")
                bn_apply(t[:], xs[g][:], A_t, B_t, g, 0.2)
            outs.append(t)
        return outs

    xB_h, stBh = evacB(psB_h, "h")
    stsH = allreduceB(stBh, "h")
    xB_w, stBw = evacB(psB_w, "w")
    stsW = allreduceB(stBw, "w")
    # critical chain first (emission order = Tile priority): unblock conv3-h
    A_Bh, B_Bh = bn_coeffs(stsH, 2, invNB, gB[:, 0:2], beB[:, 0:2], "Bsth")
    h2 = applyB(xB_h, A_Bh, B_Bh, "h")
    # c-path stage-A convs are filler for the AllReduce windows: emitted at
    # lower priority than the BN chains they would otherwise delay
    xA_c = stageA_conv(wA_c, 3, 4, 1, 2, "c", g_range=range(0, 2))
    A_Bw, B_Bw = bn_coeffs(stsW, 2, invNB, gB[:, 2:4], beB[:, 2:4], "Bstw")
    w2 = applyB(xB_w, A_Bw, B_Bw, "w")
    xA_c += stageA_conv(wA_c, 3, 4, 1, 2, "c2", g_range=range(2, 4))
    mvc = mvA[:].rearrange("p (c two) -> p c two", two=2)[:, 2:6, :]
    A_Ac, B_Ac = bn_from_mv(mvc[:, :, 0:1].squeeze(2), mvc[:, :, 1:2].squeeze(2),
                            4, gA[:, 2:6], beA[:, 2:6], "Ac")
    nc.vector.tensor_tensor(A_Ac[:], A_Ac[:], coef_p, Alu.mult)
    nc.vector.tensor_tensor(B_Ac[:], B_Ac[:], coef_p, Alu.mult)
    cT = []
    for g in range(4):
        ct = sb.tile([128, 3 * BC], F32, name=f"cT{g}")
        src = xA_c[g][:].rearrange("p (c b) -> p c b", b=BC)
        dstv = ct[:].rearrange("p (b c) -> p c b", c=3)
        bn_apply(dstv, src, A_Ac, B_Ac, g, 0.2)
        cT.append(ct)

    # ======================= stage C convs (31 -> 32) =====================
    def stageC(wtile, src_chunks, b3t, name, out_dt):
        pss = convT(wtile, src_chunks, 31, 32, 2, 4, name,
                    widx=lambda ck, k, g: ((k * 2 + ck) * 4 + g) * 128)
        outs = []
        for g in range(4):
            h = sb.tile([128, BC * 32], out_dt, name=f"{name}3_{g}")
            for bh in range(2):
                ps = pss[(g, bh)]
                # psum cols t*16+j -> sbuf cols (bh*16+j)*32 + t
                inv = ps[:].rearrange("p (t j) -> p j t", j=16)
                dstv = h[:].rearrange("p (b t) -> p b t", t=32)[
                    :, bh * 16:(bh + 1) * 16, :]
                nc.scalar.activation(dstv, inv, Act.Tanh, bias=b3t[:, g:g + 1])
            outs.append(h)
        return outs

    h3 = stageC(wC_h, h2, b3h, "h", F32)
    GDTt = GDT
    w3 = stageC(wC_w, w2, b3w, "w", GDTt)

    # ======================= rank-weighted outer-product head =============
    # G[q][r, b*96 + ci*32 + hi] = c~[q][r, b*3+ci] * h3[q][r, b*32+hi]
    Gt = []
    for q in range(4):
        g = sb.tile([128, BC * 96], GDTt, name=f"G{q}")
        cv = cT[q][:].rearrange("p (b c) -> p b c", c=3).unsqueeze(3) \
            .broadcast_to([128, BC, 3, 32])
        hv = h3[q][:].rearrange("p (b h) -> p b h", h=32).unsqueeze(2) \
            .broadcast_to([128, BC, 3, 32])
        gv = g[:].rearrange("p (b c h) -> p b c h", c=3, h=32)
        eng = nc.gpsimd if q == 3 else nc.vector
        half = BC // 2
        for hh in range(2):
            eng.tensor_tensor(gv[:, hh * half:(hh + 1) * half],
                              cv[:, hh * half:(hh + 1) * half],
                              hv[:, hh * half:(hh + 1) * half], Alu.mult)
        Gt.append(g)

    outsb = sb.tile([96, BC * 32], F32, name="outsb")
    out_v = out_ap.rearrange("b c h w -> (c h) b w")
    for grp in range(BC // 4):
        po = psp.tile([96, 4 * 32], F32, name=f"po{grp}", tag="ps")
        for q in range(4):
            for j in range(4):
                bb = grp * 4 + j
                nc.tensor.matmul(po[:, j * 32:(j + 1) * 32],
                                 Gt[q][:, bb * 96:(bb + 1) * 96],
                                 w3[q][:, bb * 32:(bb + 1) * 32],
                                 start=(j == 0 and q == 0),
                                 stop=(j == 3 and q == 3))
        nc.vector.tensor_copy(outsb[:, grp * 128:(grp + 1) * 128], po[:])
        if grp % 4 == 3:
            # stream each 16-sample half out as soon as it is evacuated
            h0 = (grp - 3) * 4
            nc.sync.dma_start(
                out_v[:, h0:h0 + 16, :],
                outsb[:, h0 * 32:(h0 + 16) * 32].rearrange("p (b w) -> p b w", w=32))

    sb.release()
    psp.release()
    dram.release()


# --------------------------------------------------------------------------
# host side
# --------------------------------------------------------------------------

def _build_module():
    nc = bacc.Bacc("TRN2", target_bir_lowering=False, debug=False,
                   num_devices=N_CORES)
    WDT = _cdt()
    specs = {
        "p100": ((NOISE, 128 + B), F32),
        "p10": ((NCLASS, B + 128), F32),
        "p128": ((128, 34), F32),
        "wA_c": ((128, 3072), WDT), "wA_h": ((128, 4096), WDT),
        "wA_w": ((128, 4096), WDT),
        "wB_h": ((128, 4096), WDT), "wB_w": ((128, 4096), WDT),
        "wC_h": ((128, 2048), WDT), "wC_w": ((128, 2048), WDT),
    }
    I = {}
    for name, (shape, dt) in specs.items():
        I[name] = nc.dram_tensor(name, list(shape), dt, kind="ExternalInput").ap()
    out = nc.dram_tensor("out", [BC, 3, 32, 32], F32, kind="ExternalOutput")
    with tile.TileContext(nc) as tc:
        _emit(nc, tc, I, out.ap())
    nc.compile()
    return nc


def _np(x):
    return np.ascontiguousarray(np.asarray(x, dtype=np.float32))


def _pack_inputs(inputs):
    """host-side layout packing -> (replicated dict, per-core dicts)"""
    wnp = np.dtype(mybir.dt.np(_cdt()))
    noise = _np(inputs["noise"])
    label = np.asarray(inputs["label"]).astype(np.int64)

    c_w1 = _np(inputs["c_w1"])   # (256, 512, 3)
    h_w1 = _np(inputs["h_w1"])   # (256, 128, 16)
    w_w1 = _np(inputs["w_w1"])
    h_w2 = _np(inputs["h_w2"])   # (128, 256, 16)
    w_w2 = _np(inputs["w_w2"])
    h_w3 = _np(inputs["h_w3"])   # (256, 512, 2)
    w_w3 = _np(inputs["w_w3"])

    def packA_c(w):   # -> [ci_in, (k, chunk, g, co_in)]
        return np.ascontiguousarray(
            w.reshape(2, 128, 4, 128, 3).transpose(1, 4, 0, 2, 3).reshape(128, -1))

    def packA_h(w):   # (256,128,16) -> [ci_in, (k, chunk, co)]
        return np.ascontiguousarray(
            w.reshape(2, 128, 128, 16).transpose(1, 3, 0, 2).reshape(128, -1))

    def packB(w):     # (128,256,16) -> [ci, (k, g, co_in)]
        return np.ascontiguousarray(
            w.reshape(128, 2, 128, 16).transpose(0, 3, 1, 2).reshape(128, -1))

    def packC(w):     # (256,512,2) -> [ci_in, (k, chunk, g, co_in)]
        return np.ascontiguousarray(
            w.reshape(2, 128, 4, 128, 2).transpose(1, 4, 0, 2, 3).reshape(128, -1))

    def col128(*arrs):
        return np.ascontiguousarray(
            np.concatenate([a.reshape(-1, 128).T for a in arrs], axis=1))

    p128 = np.concatenate([
        _np(inputs["bn0_g"]).reshape(128, 1),
        _np(inputs["bn0_b"]).reshape(128, 1),
        col128(_np(inputs["h_g1"]), _np(inputs["w_g1"]), _np(inputs["c_g1"])),
        col128(_np(inputs["h_be1"]), _np(inputs["w_be1"]), _np(inputs["c_be1"])),
        col128(_np(inputs["coef"])),
        col128(_np(inputs["h_g2"]), _np(inputs["w_g2"])),
        col128(_np(inputs["h_be2"]), _np(inputs["w_be2"])),
        col128(_np(inputs["h_b3"])),
        col128(_np(inputs["w_b3"])),
    ], axis=1)

    rep = {
        "p128": np.ascontiguousarray(p128),
        "wA_c": packA_c(c_w1).astype(wnp),
        "wA_h": packA_h(h_w1).astype(wnp),
        "wA_w": packA_h(w_w1).astype(wnp),
        "wB_h": packB(h_w2).astype(wnp),
        "wB_w": packB(w_w2).astype(wnp),
        "wC_h": packC(h_w3).astype(wnp),
        "wC_w": packC(w_w3).astype(wnp),
    }

    lin_wt = _np(inputs["lin_w"]).T
    emb_w = _np(inputs["emb"])
    noise_t = noise.T
    per_core = []
    for c in range(N_CORES):
        own = np.arange(c * BC, (c + 1) * BC)
        rest = np.delete(np.arange(B), own)
        perm = np.concatenate([own, rest])
        oh = (label[perm][None, :] == np.arange(NCLASS)[:, None]).astype(np.float32)
        per_core.append({
            "p100": np.ascontiguousarray(
                np.concatenate([lin_wt, noise_t[:, perm]], axis=1)),
            "p10": np.ascontiguousarray(np.concatenate([oh, emb_w], axis=1)),
            **rep,
        })
    return per_core


def kernel(**inputs) -> np.ndarray:
    with _lock:
        key = (PREC, NO_AR, SIM_SAFE)
        nc = _cache.get(key)
        if nc is None:
            nc = _build_module()
            _cache[key] = nc
    in_maps = _pack_inputs(inputs)
    res = run_bass_kernel_spmd(nc, in_maps, core_ids=list(range(N_CORES)))
    return np.concatenate([r["out"] for r in res.results], axis=0)
